# revision 21
# baseline (speedup 1.0000x reference)
"""Trainium2 Bass kernel for nn_MixtureCogrammar.

Computation (reference):
    attn  = softmax(morphosyn @ W_affix)                    [B, V]
    affix = attn @ affix_vocab.reshape(V, D*N)              [B, D, N]
    wC    = cumsum_n( sum_{ijk} a_i b_j f_k softmax(pivot_logits[i,j,:,k,:]) )
    out   = stem + wC * (affix - stem)

Distribution (v7): 2x4 grid — batch split 2 ways, D split 4 ways.
Per core: stem/out [512, 64, N] bf16 (16.8MB each), vocab [V, 64, N] fp8
(8.4MB), pivot for 512 batches (5.2MB bf16). wC for all 4 local batch
chunks is computed LOCALLY — no collective. (The gpsimd AllGather of the
pure D-sharded variant measured a fixed ~55us CC-handshake latency that
froze the wC-gated pipeline; the pure B-sharded variant paid 33.5MB of
replicated vocab DMA. This grid pays 8.4MB vocab and zero collectives.)

Kernel structure (from trace analysis of v2-v5):
  - fp8(e4m3) matmul in DoubleRow perf mode; attn scaled x128 on device
    and stored fp8 (measured ~263ns per 512-wide DR matmul).
  - stem subtracted IN PSUM via an identity matmul (lhsT = -128*I fp8,
    rhs = resident bf16 stem tile): psum = 128*(affix - stem), so the
    elementwise tail is only
      ScalarE: delta = psum * 1/128      (PSUM->SBUF bf16 drain)
      DVE:     prod  = delta * wC        (bf16 2x)
      DVE:     out   = prod + stem       (bf16 2x)
  - d-block outer / batch-chunk inner loop with 8-16KB per-partition DMA
    lines (DMA engines are packet-overhead limited).
  - pivot path per chunk: 20 exps with accumulate on ScalarE, weighted
    bf16 chain + scan on DVE; runs ahead of the main loop.
"""

import os
import sys

import numpy as np

for _p in ("/opt/trn_rl_repo",):
    if os.path.isdir(_p) and _p not in sys.path:
        sys.path.append(_p)

import concourse.bass as bass  # noqa: E402
import concourse.tile as tile  # noqa: E402
from concourse import bacc, mybir  # noqa: E402
from concourse.bass import ts  # noqa: E402
from concourse.bass_utils import run_bass_kernel_spmd  # noqa: E402
from concourse.masks import make_identity  # noqa: E402

import ml_dtypes  # noqa: E402

B, D, N, DM, V = 1024, 256, 256, 128, 512
NCORES = 8
NBG, NDG = 2, 4             # core grid: batch groups x d groups
BLOC = B // NBG             # 512 batches per core
NBC = BLOC // 128           # 4 local batch chunks
DLOC = D // NDG             # 64 d-values per core
NDB = 2                     # d-blocks per core
DB = DLOC // NDB            # 32 d-values per block
DN = DB * N                 # 8192 free elems per (chunk, d-block) tile
PSW = 2048                  # psum tile = 4 banks
NT = DN // PSW              # 4 h-tiles
SCALE = 128.0               # attn pre-scale so fp8 subnormals don't bite

F32 = mybir.dt.float32
F32R = mybir.dt.float32r
BF16 = mybir.dt.bfloat16
FP8 = mybir.dt.float8e4
EXP = mybir.ActivationFunctionType.Exp
COPY = mybir.ActivationFunctionType.Copy
ALU = mybir.AluOpType
DR = mybir.MatmulPerfMode.DoubleRow

LAST_RESULT = None

_CACHE = {}


def _build():
    key = 0
    if key in _CACHE:
        return _CACHE[key]

    nc = bacc.Bacc("TRN2", target_bir_lowering=False, debug=False,
                   num_devices=NCORES)

    stem_d = nc.dram_tensor("stem", [BLOC, DLOC, N], BF16, kind="ExternalInput").ap()
    vocab_d = nc.dram_tensor("vocab", [V, DLOC, N], FP8, kind="ExternalInput").ap()
    mor_d = nc.dram_tensor("morpho", [BLOC, DM], F32, kind="ExternalInput").ap()
    waff_d = nc.dram_tensor("waffix", [DM, V], F32R, kind="ExternalInput").ap()
    pv_d = nc.dram_tensor("pivot", [2, 2, BLOC, 5, N], BF16, kind="ExternalInput").ap()
    abf_d = nc.dram_tensor("abf", [1, 9], F32, kind="ExternalInput").ap()
    out_d = nc.dram_tensor("out", [BLOC, DLOC, N], BF16, kind="ExternalOutput").ap()

    from contextlib import ExitStack

    with tile.TileContext(nc) as tc, ExitStack() as ctx:
        const = ctx.enter_context(tc.tile_pool(name="const", bufs=1))

        ident = const.tile([128, 128], F32)
        make_identity(nc, ident[:, :])
        negI = const.tile([128, 128], FP8)
        nc.scalar.mul(negI[:, :], ident[:, :], -SCALE)

        attnT = const.tile([128, 4, BLOC], FP8)    # [v_part, vc, local b]
        w_bcast = const.tile([128, 20], F32)
        wsb = const.tile([128, V], F32R)           # W_affix resident
        mor_sb = const.tile([128, NBC, DM], F32)
        wc_sb = const.tile([128, NBC, N], BF16)    # local wC per chunk

        # ---------- DMAs: wC path on the Activation HWDGE queue ----------
        small = ctx.enter_context(tc.tile_pool(name="small", bufs=1))
        pvp = ctx.enter_context(tc.tile_pool(name="pv", bufs=1))
        abf = small.tile([1, 9], F32)
        nc.scalar.dma_start(abf[0:1, :], abf_d[:, :])
        pvt = []
        for bc in range(NBC):
            pvc = pvp.tile([128, 4, 5, N], BF16, tag="pvin", name=f"pv{bc}")
            for ij in range(4):
                i, j = divmod(ij, 2)
                nc.scalar.dma_start(pvc[:, ij, :, :],
                                    pv_d[i, j, ts(bc, 128), :, :])
            pvt.append(pvc)
        for bc in range(NBC):
            nc.scalar.dma_start(mor_sb[:, bc, :], mor_d[ts(bc, 128), :])
        nc.scalar.dma_start(wsb[:, :], waff_d[:, :])

        # ---------- mixture weights ----------
        eabf = small.tile([1, 9], F32)
        sums = small.tile([1, 3], F32)
        nc.scalar.activation(eabf[0:1, 0:2], abf[0:1, 0:2], EXP, accum_out=sums[0:1, 0:1])
        nc.scalar.activation(eabf[0:1, 2:4], abf[0:1, 2:4], EXP, accum_out=sums[0:1, 1:2])
        nc.scalar.activation(eabf[0:1, 4:9], abf[0:1, 4:9], EXP, accum_out=sums[0:1, 2:3])
        rsum = small.tile([1, 3], F32)
        nc.vector.reciprocal(rsum[0:1, :], sums[0:1, :])
        t4 = small.tile([1, 4], F32)
        nc.vector.tensor_mul(
            t4[0:1, :].rearrange("p (i j) -> p i j", i=2),
            eabf[0:1, 0:2].rearrange("p (i j) -> p i j", j=1).to_broadcast((1, 2, 2)),
            eabf[0:1, 2:4].rearrange("p (i j) -> p i j", i=1).to_broadcast((1, 2, 2)),
        )
        t20 = small.tile([1, 20], F32)
        nc.vector.tensor_mul(
            t20[0:1, :].rearrange("p (g k) -> p g k", g=4),
            t4[0:1, :].rearrange("p (g k) -> p g k", k=1).to_broadcast((1, 4, 5)),
            eabf[0:1, 4:9].rearrange("p (g k) -> p g k", g=1).to_broadcast((1, 4, 5)),
        )
        rr = small.tile([1, 1], F32)
        nc.vector.tensor_mul(rr[0:1, :], rsum[0:1, 0:1], rsum[0:1, 1:2])
        rrr = small.tile([1, 1], F32)
        nc.vector.tensor_mul(rrr[0:1, :], rr[0:1, :], rsum[0:1, 2:3])
        w20 = small.tile([1, 20], F32)
        nc.vector.tensor_scalar_mul(w20[0:1, :], t20[0:1, :], rrr[0:1, 0:1])
        nc.gpsimd.partition_broadcast(w_bcast[:, :], w20[0:1, :])

        # pivot softmax+mix for one chunk; emitted per-chunk, interleaved
        # with the main loop (needs no PSUM, so it runs behind the MMs)
        def pivot_chunk(bc):
            pvc = pvt[bc]
            pvE = pvp.tile([128, 20, N], BF16, tag="pvE", name=f"pvE{bc}")
            nc.scalar.activation(
                pvE[:, :, :].rearrange("p (ij k) n -> p ij k n", ij=4),
                pvc[:, :, :, :], EXP)
            # group sums: TT-add tree (2x-capable) + small reduce (the direct
            # [128,20,256] TENSOR_REDUCE measured 1x = 5.5us)
            t1 = pvp.tile([128, 20, 128], BF16, tag="t1", name=f"t1{bc}")
            nc.vector.tensor_add(t1[:, :, :], pvE[:, :, 0:128], pvE[:, :, 128:256])
            t2 = pvp.tile([128, 20, 64], BF16, tag="t2", name=f"t2{bc}")
            nc.vector.tensor_add(t2[:, :, :], t1[:, :, 0:64], t1[:, :, 64:128])
            sP = pvp.tile([128, 20, 1], F32, tag="sP", name=f"sP{bc}")
            nc.vector.reduce_sum(sP[:, :, :], t2[:, :, :],
                                 axis=mybir.AxisListType.X)
            rP = pvp.tile([128, 20], F32, tag="rP", name=f"rP{bc}")
            nc.vector.reciprocal(rP[:, :], sP[:, :, 0])
            rPw = pvp.tile([128, 20], F32, tag="rPw", name=f"rPw{bc}")
            nc.vector.tensor_mul(rPw[:, :], rP[:, :], w_bcast[:, :])
            # weighted accumulate over the 20 groups on GpSimd (DVE was the
            # pipeline pacer; STT runs 1x there anyway)
            accA = pvp.tile([128, N], BF16, tag="accA", name=f"aA{bc}")
            accB = pvp.tile([128, N], BF16, tag="accB", name=f"aB{bc}")
            tmp = pvp.tile([128, N], BF16, tag="tmp", name=f"tm{bc}")
            nc.gpsimd.tensor_mul(accA[:, :], pvE[:, 0, :],
                                 rPw[:, 0:1].to_broadcast((128, N)))
            cur, nxt = accA, accB
            for g in range(1, 20):
                nc.gpsimd.tensor_mul(tmp[:, :], pvE[:, g, :],
                                     rPw[:, g:g + 1].to_broadcast((128, N)))
                nc.gpsimd.tensor_add(nxt[:, :], tmp[:, :], cur[:, :])
                cur, nxt = nxt, cur
            nc.vector.tensor_tensor_scan(
                wc_sb[:, bc, :], data0=cur[:, :], data1=cur[:, :], initial=0.0,
                op0=ALU.add, op1=ALU.bypass,
            )

        # ---------- attention (4 local chunks) ----------
        bp = ctx.enter_context(tc.tile_pool(name="attn", bufs=2))
        psB = tc.alloc_tile_pool(name="psB", bufs=2, space="PSUM")
        psT = tc.alloc_tile_pool(name="psT", bufs=1, space="PSUM")

        for bc in range(NBC):
            morT_ps = psB.tile([128, DM], F32, tag="morT_ps", name=f"mtp{bc}")
            nc.tensor.transpose(morT_ps[:, :], mor_sb[:, bc, :], ident[:, :])
            morT = bp.tile([128, DM], F32R, tag="morT", name=f"mt{bc}")
            nc.vector.tensor_copy(morT[:, :], morT_ps[:, :])
            lg_ps = psB.tile([128, V], F32, tag="lg_ps", name=f"lgp{bc}")
            nc.tensor.matmul(lg_ps[:, :], lhsT=morT[:, :], rhs=wsb[:, :],
                             start=True, stop=True)
            E = bp.tile([128, V], F32, tag="E", name=f"E{bc}")
            sE = bp.tile([128, 1], F32, tag="sE", name=f"sE{bc}")
            nc.scalar.activation(E[:, :], lg_ps[:, :], EXP, accum_out=sE[:, :])
            rE = bp.tile([128, 1], F32, tag="rE", name=f"rE{bc}")
            nc.vector.reciprocal(rE[:, :], sE[:, :])
            rEs = bp.tile([128, 1], F32, tag="rEs", name=f"rs{bc}")
            nc.vector.tensor_scalar_mul(rEs[:, :], rE[:, :], SCALE)
            attn = bp.tile([128, V], F32, tag="at", name=f"at{bc}")
            nc.vector.tensor_scalar_mul(attn[:, :], E[:, :], rEs[:, 0:1])
            tp = psT.tile([128, 4, 512], F32, tag="tp", name=f"tp{bc}")
            for vc in range(4):
                nc.tensor.transpose(tp[:, vc, 0:128], attn[:, ts(vc, 128)],
                                    ident[:, :])
            nc.scalar.copy(attnT[:, :, ts(bc, 128)], tp[:, :, 0:128])
        psT.release()
        psB.release()

        # ---------- main loop: d-block outer, batch-chunk inner ----------
        vqp = ctx.enter_context(tc.tile_pool(name="vq", bufs=2))
        stp = ctx.enter_context(tc.tile_pool(name="stem", bufs=2))
        otp = ctx.enter_context(tc.tile_pool(name="outp", bufs=2))
        dlp = ctx.enter_context(tc.tile_pool(name="delta", bufs=5))
        prp = ctx.enter_context(tc.tile_pool(name="prod", bufs=2))
        psD = ctx.enter_context(tc.tile_pool(name="psD", bufs=2, space="PSUM"))

        for db in range(NDB):
            vq = vqp.tile([128, 4, DN], FP8)
            for vc in range(4):
                nc.sync.dma_start(
                    vq[:, vc, :],
                    vocab_d[ts(vc, 128), ts(db, DB), :].rearrange("p d n -> p (d n)"),
                )
            for bc in range(NBC):
                if db == 0:
                    pivot_chunk(bc)
                stem_t = stp.tile([128, DN], BF16)
                nc.sync.dma_start(
                    stem_t[:, :],
                    stem_d[ts(bc, 128), ts(db, DB), :].rearrange("p d n -> p (d n)"),
                )
                out_t = otp.tile([128, DN], BF16)
                for h in range(NT):
                    col0 = h * PSW
                    ps = psD.tile([128, PSW], F32)
                    pseq = (0, 1) if h % 2 == 0 else (1, 0)
                    for pi, p in enumerate(pseq):
                        for t in range(PSW // 512):
                            c = col0 + t * 512
                            nc.tensor.matmul(
                                ps[:, ts(t, 512)],
                                lhsT=attnT[:, 2 * p:2 * p + 2, ts(bc, 128)],
                                rhs=vq[:, 2 * p:2 * p + 2, c:c + 512],
                                start=(pi == 0), stop=False,
                                perf_mode=DR,
                            )
                    # subtract SCALE*stem in PSUM: psum = SCALE*(affix-stem)
                    for t in range(PSW // 512):
                        c = col0 + t * 512
                        nc.tensor.matmul(
                            ps[:, ts(t, 512)],
                            lhsT=negI[:, :],
                            rhs=stem_t[:, c:c + 512],
                            start=False, stop=True,
                        )
                    delta_t = dlp.tile([128, PSW], BF16)
                    nc.scalar.activation(delta_t[:, :], ps[:, :], COPY,
                                         scale=1.0 / SCALE)
                    prod = prp.tile([128, PSW], BF16)
                    nc.vector.tensor_mul(
                        prod[:, :].rearrange("p (a n) -> p a n", n=N),
                        delta_t[:, :].rearrange("p (a n) -> p a n", n=N),
                        wc_sb[:, bc:bc + 1, :].to_broadcast((128, PSW // N, N)),
                    )
                    nc.vector.tensor_add(out_t[:, col0:col0 + PSW], prod[:, :],
                                         stem_t[:, col0:col0 + PSW])
                nc.sync.dma_start(
                    out_d[ts(bc, 128), ts(db, DB), :].rearrange("p d n -> p (d n)"),
                    out_t[:, :],
                )

    nc.compile()
    _CACHE[key] = nc
    return nc


def kernel(stem_form, morphosyn, pivot_logits, W_affix, affix_vocab,
           alpha, beta, phi, max_len):
    global LAST_RESULT
    stem_form = np.ascontiguousarray(np.asarray(stem_form, dtype=np.float32))
    morphosyn = np.ascontiguousarray(np.asarray(morphosyn, dtype=np.float32))
    pivot_logits = np.ascontiguousarray(np.asarray(pivot_logits, dtype=np.float32))
    W_affix = np.ascontiguousarray(np.asarray(W_affix, dtype=np.float32))
    affix_vocab = np.ascontiguousarray(np.asarray(affix_vocab, dtype=np.float32))
    abf = np.concatenate([
        np.asarray(alpha, np.float32).ravel(),
        np.asarray(beta, np.float32).ravel(),
        np.asarray(phi, np.float32).ravel(),
    ]).reshape(1, 9)

    nc = _build()

    stem_np = stem_form.astype(ml_dtypes.bfloat16)
    vocab_np = affix_vocab.astype(ml_dtypes.float8_e4m3)
    pivot_np = pivot_logits.astype(ml_dtypes.bfloat16)

    in_maps = []
    for c in range(NCORES):
        bg, dg = divmod(c, NDG)
        blo, bhi = bg * BLOC, (bg + 1) * BLOC
        dlo, dhi = dg * DLOC, (dg + 1) * DLOC
        in_maps.append({
            "stem": np.ascontiguousarray(stem_np[blo:bhi, dlo:dhi, :]),
            "vocab": np.ascontiguousarray(vocab_np[:, dlo:dhi, :]),
            "morpho": np.ascontiguousarray(morphosyn[blo:bhi]),
            "waffix": W_affix,
            "pivot": np.ascontiguousarray(pivot_np[:, :, blo:bhi, :, :]),
            "abf": abf,
        })

    LAST_RESULT = run_bass_kernel_spmd(nc, in_maps, core_ids=list(range(NCORES)))
    out = np.empty((B, D, N), dtype=np.float32)
    for c in range(NCORES):
        bg, dg = divmod(c, NDG)
        out[bg * BLOC:(bg + 1) * BLOC, dg * DLOC:(dg + 1) * DLOC, :] = \
            LAST_RESULT.results[c]["out"].astype(np.float32)
    return np.ascontiguousarray(out)


# revision 24
# speedup vs baseline: 1.3735x; 1.3735x over previous
"""Trainium2 Bass kernel for nn_MixtureCogrammar.

Computation (reference):
    attn  = softmax(morphosyn @ W_affix)                    [B, V]
    affix = attn @ affix_vocab.reshape(V, D*N)              [B, D, N]
    wC    = cumsum_n( sum_{ijk} a_i b_j f_k softmax(pivot_logits[i,j,:,k,:]) )
    out   = stem + wC * (affix - stem)

Distribution (v7): 2x4 grid — batch split 2 ways, D split 4 ways.
Per core: stem/out [512, 64, N] bf16 (16.8MB each), vocab [V, 64, N] fp8
(8.4MB), pivot for 512 batches (5.2MB bf16). wC for all 4 local batch
chunks is computed LOCALLY — no collective. (The gpsimd AllGather of the
pure D-sharded variant measured a fixed ~55us CC-handshake latency that
froze the wC-gated pipeline; the pure B-sharded variant paid 33.5MB of
replicated vocab DMA. This grid pays 8.4MB vocab and zero collectives.)

Kernel structure (from trace analysis of v2-v5):
  - fp8(e4m3) matmul in DoubleRow perf mode; attn scaled x128 on device
    and stored fp8 (measured ~263ns per 512-wide DR matmul).
  - stem subtracted IN PSUM via an identity matmul (lhsT = -128*I fp8,
    rhs = resident bf16 stem tile): psum = 128*(affix - stem), so the
    elementwise tail is only
      ScalarE: delta = psum * 1/128      (PSUM->SBUF bf16 drain)
      DVE:     prod  = delta * wC        (bf16 2x)
      DVE:     out   = prod + stem       (bf16 2x)
  - d-block outer / batch-chunk inner loop with 8-16KB per-partition DMA
    lines (DMA engines are packet-overhead limited).
  - pivot path per chunk: 20 exps with accumulate on ScalarE, weighted
    bf16 chain + scan on DVE; runs ahead of the main loop.
"""

import os
import sys

import numpy as np

for _p in ("/opt/trn_rl_repo",):
    if os.path.isdir(_p) and _p not in sys.path:
        sys.path.append(_p)

import concourse.bass as bass  # noqa: E402
import concourse.tile as tile  # noqa: E402
from concourse import bacc, mybir  # noqa: E402
from concourse.bass import ts  # noqa: E402
from concourse.bass_utils import run_bass_kernel_spmd  # noqa: E402
from concourse.masks import make_identity  # noqa: E402

import ml_dtypes  # noqa: E402

B, D, N, DM, V = 1024, 256, 256, 128, 512
NCORES = 8
NBG, NDG = 2, 4             # core grid: batch groups x d groups
BLOC = B // NBG             # 512 batches per core
NBC = BLOC // 128           # 4 local batch chunks
DLOC = D // NDG             # 64 d-values per core
NDB = 2                     # d-blocks per core
DB = DLOC // NDB            # 32 d-values per block
DN = DB * N                 # 8192 free elems per (chunk, d-block) tile
PSW = 2048                  # psum tile = 4 banks
NT = DN // PSW              # 4 h-tiles
SCALE = 128.0               # attn pre-scale so fp8 subnormals don't bite

F32 = mybir.dt.float32
F32R = mybir.dt.float32r
BF16 = mybir.dt.bfloat16
FP8 = mybir.dt.float8e4
EXP = mybir.ActivationFunctionType.Exp
COPY = mybir.ActivationFunctionType.Copy
ALU = mybir.AluOpType
DR = mybir.MatmulPerfMode.DoubleRow

LAST_RESULT = None

_CACHE = {}


def _build():
    key = 0
    if key in _CACHE:
        return _CACHE[key]

    nc = bacc.Bacc("TRN2", target_bir_lowering=False, debug=False,
                   num_devices=NCORES)

    stem_d = nc.dram_tensor("stem", [BLOC, DLOC, N], BF16, kind="ExternalInput").ap()
    vocab_d = nc.dram_tensor("vocab", [V, DLOC, N], FP8, kind="ExternalInput").ap()
    mor_d = nc.dram_tensor("morpho", [BLOC, DM], F32, kind="ExternalInput").ap()
    waff_d = nc.dram_tensor("waffix", [DM, V], F32R, kind="ExternalInput").ap()
    pv_d = nc.dram_tensor("pivot", [2, 2, BLOC, 5, N], BF16, kind="ExternalInput").ap()
    abf_d = nc.dram_tensor("abf", [1, 9], F32, kind="ExternalInput").ap()
    out_d = nc.dram_tensor("out", [BLOC, DLOC, N], BF16, kind="ExternalOutput").ap()

    from contextlib import ExitStack

    with tile.TileContext(nc) as tc, ExitStack() as ctx:
        const = ctx.enter_context(tc.tile_pool(name="const", bufs=1))

        ident = const.tile([128, 128], F32)
        make_identity(nc, ident[:, :])
        negI = const.tile([128, 128], FP8)
        nc.scalar.mul(negI[:, :], ident[:, :], -SCALE)

        attnT = const.tile([128, 4, BLOC], FP8)    # [v_part, vc, local b]
        w_bcast = const.tile([128, 20], F32)
        wsb = const.tile([128, V], F32R)           # W_affix resident
        mor_sb = const.tile([128, NBC, DM], F32)
        wc_sb = const.tile([128, NBC, N], BF16)    # local wC per chunk

        # ---------- DMAs: wC path on the Activation HWDGE queue ----------
        small = ctx.enter_context(tc.tile_pool(name="small", bufs=1))
        pvp = ctx.enter_context(tc.tile_pool(name="pv", bufs=1))
        abf = small.tile([1, 9], F32)
        nc.scalar.dma_start(abf[0:1, :], abf_d[:, :])
        pvt = []
        for bc in range(NBC):
            pvc = pvp.tile([128, 4, 5, N], BF16, tag="pvin", name=f"pv{bc}")
            for ij in range(4):
                i, j = divmod(ij, 2)
                nc.scalar.dma_start(pvc[:, ij, :, :],
                                    pv_d[i, j, ts(bc, 128), :, :])
            pvt.append(pvc)
        for bc in range(NBC):
            nc.scalar.dma_start(mor_sb[:, bc, :], mor_d[ts(bc, 128), :])
        nc.scalar.dma_start(wsb[:, :], waff_d[:, :])

        # ---------- mixture weights ----------
        eabf = small.tile([1, 9], F32)
        sums = small.tile([1, 3], F32)
        nc.scalar.activation(eabf[0:1, 0:2], abf[0:1, 0:2], EXP, accum_out=sums[0:1, 0:1])
        nc.scalar.activation(eabf[0:1, 2:4], abf[0:1, 2:4], EXP, accum_out=sums[0:1, 1:2])
        nc.scalar.activation(eabf[0:1, 4:9], abf[0:1, 4:9], EXP, accum_out=sums[0:1, 2:3])
        rsum = small.tile([1, 3], F32)
        nc.vector.reciprocal(rsum[0:1, :], sums[0:1, :])
        t4 = small.tile([1, 4], F32)
        nc.vector.tensor_mul(
            t4[0:1, :].rearrange("p (i j) -> p i j", i=2),
            eabf[0:1, 0:2].rearrange("p (i j) -> p i j", j=1).to_broadcast((1, 2, 2)),
            eabf[0:1, 2:4].rearrange("p (i j) -> p i j", i=1).to_broadcast((1, 2, 2)),
        )
        t20 = small.tile([1, 20], F32)
        nc.vector.tensor_mul(
            t20[0:1, :].rearrange("p (g k) -> p g k", g=4),
            t4[0:1, :].rearrange("p (g k) -> p g k", k=1).to_broadcast((1, 4, 5)),
            eabf[0:1, 4:9].rearrange("p (g k) -> p g k", g=1).to_broadcast((1, 4, 5)),
        )
        rr = small.tile([1, 1], F32)
        nc.vector.tensor_mul(rr[0:1, :], rsum[0:1, 0:1], rsum[0:1, 1:2])
        rrr = small.tile([1, 1], F32)
        nc.vector.tensor_mul(rrr[0:1, :], rr[0:1, :], rsum[0:1, 2:3])
        w20 = small.tile([1, 20], F32)
        nc.vector.tensor_scalar_mul(w20[0:1, :], t20[0:1, :], rrr[0:1, 0:1])
        nc.gpsimd.partition_broadcast(w_bcast[:, :], w20[0:1, :])

        # pivot softmax+mix for one chunk; emitted per-chunk, interleaved
        # with the main loop (needs no PSUM, so it runs behind the MMs)
        def pivot_chunk(bc):
            pvc = pvt[bc]
            pvE = pvp.tile([128, 20, N], BF16, tag="pvE", name=f"pvE{bc}")
            nc.scalar.activation(
                pvE[:, :, :].rearrange("p (ij k) n -> p ij k n", ij=4),
                pvc[:, :, :, :], EXP)
            # group sums: TT-add tree (2x-capable) + small reduce (the direct
            # [128,20,256] TENSOR_REDUCE measured 1x = 5.5us)
            t1 = pvp.tile([128, 20, 128], BF16, tag="t1", name=f"t1{bc}")
            nc.vector.tensor_add(t1[:, :, :], pvE[:, :, 0:128], pvE[:, :, 128:256])
            t2 = pvp.tile([128, 20, 64], BF16, tag="t2", name=f"t2{bc}")
            nc.vector.tensor_add(t2[:, :, :], t1[:, :, 0:64], t1[:, :, 64:128])
            sP = pvp.tile([128, 20, 1], F32, tag="sP", name=f"sP{bc}")
            nc.vector.reduce_sum(sP[:, :, :], t2[:, :, :],
                                 axis=mybir.AxisListType.X)
            rP = pvp.tile([128, 20], F32, tag="rP", name=f"rP{bc}")
            nc.vector.reciprocal(rP[:, :], sP[:, :, 0])
            rPw = pvp.tile([128, 20], F32, tag="rPw", name=f"rPw{bc}")
            nc.vector.tensor_mul(rPw[:, :], rP[:, :], w_bcast[:, :])
            accA = pvp.tile([128, N], BF16, tag="accA", name=f"aA{bc}")
            accB = pvp.tile([128, N], BF16, tag="accB", name=f"aB{bc}")
            nc.vector.tensor_scalar_mul(accA[:, :], pvE[:, 0, :], rPw[:, 0:1])
            cur, nxt = accA, accB
            for g in range(1, 20):
                nc.vector.scalar_tensor_tensor(
                    out=nxt[:, :], in0=pvE[:, g, :], scalar=rPw[:, g:g + 1],
                    in1=cur[:, :], op0=ALU.mult, op1=ALU.add,
                )
                cur, nxt = nxt, cur
            nc.vector.tensor_tensor_scan(
                wc_sb[:, bc, :], data0=cur[:, :], data1=cur[:, :], initial=0.0,
                op0=ALU.add, op1=ALU.bypass,
            )

        # ---------- attention (4 local chunks) ----------
        bp = ctx.enter_context(tc.tile_pool(name="attn", bufs=2))
        psB = tc.alloc_tile_pool(name="psB", bufs=2, space="PSUM")
        psT = tc.alloc_tile_pool(name="psT", bufs=1, space="PSUM")

        for bc in range(NBC):
            morT_ps = psB.tile([128, DM], F32, tag="morT_ps", name=f"mtp{bc}")
            nc.tensor.transpose(morT_ps[:, :], mor_sb[:, bc, :], ident[:, :])
            morT = bp.tile([128, DM], F32R, tag="morT", name=f"mt{bc}")
            nc.vector.tensor_copy(morT[:, :], morT_ps[:, :])
            lg_ps = psB.tile([128, V], F32, tag="lg_ps", name=f"lgp{bc}")
            nc.tensor.matmul(lg_ps[:, :], lhsT=morT[:, :], rhs=wsb[:, :],
                             start=True, stop=True)
            E = bp.tile([128, V], F32, tag="E", name=f"E{bc}")
            sE = bp.tile([128, 1], F32, tag="sE", name=f"sE{bc}")
            nc.scalar.activation(E[:, :], lg_ps[:, :], EXP, accum_out=sE[:, :])
            rE = bp.tile([128, 1], F32, tag="rE", name=f"rE{bc}")
            nc.vector.reciprocal(rE[:, :], sE[:, :])
            rEs = bp.tile([128, 1], F32, tag="rEs", name=f"rs{bc}")
            nc.vector.tensor_scalar_mul(rEs[:, :], rE[:, :], SCALE)
            attn = bp.tile([128, V], F32, tag="at", name=f"at{bc}")
            nc.vector.tensor_scalar_mul(attn[:, :], E[:, :], rEs[:, 0:1])
            tp = psT.tile([128, 4, 512], F32, tag="tp", name=f"tp{bc}")
            for vc in range(4):
                nc.tensor.transpose(tp[:, vc, 0:128], attn[:, ts(vc, 128)],
                                    ident[:, :])
            nc.scalar.copy(attnT[:, :, ts(bc, 128)], tp[:, :, 0:128])
        psT.release()
        psB.release()

        # ---------- main loop: d-block outer, batch-chunk inner ----------
        vqp = ctx.enter_context(tc.tile_pool(name="vq", bufs=2))
        stp = ctx.enter_context(tc.tile_pool(name="stem", bufs=2))
        otp = ctx.enter_context(tc.tile_pool(name="outp", bufs=2))
        dlp = ctx.enter_context(tc.tile_pool(name="delta", bufs=6))
        prp = ctx.enter_context(tc.tile_pool(name="prod", bufs=2))
        psD = ctx.enter_context(tc.tile_pool(name="psD", bufs=2, space="PSUM"))

        for db in range(NDB):
            vq = vqp.tile([128, 4, DN], FP8)
            for vc in range(4):
                nc.sync.dma_start(
                    vq[:, vc, :],
                    vocab_d[ts(vc, 128), ts(db, DB), :].rearrange("p d n -> p (d n)"),
                )
            for bc in range(NBC):
                if db == 0:
                    pivot_chunk(bc)
                stem_t = stp.tile([128, DN], BF16)
                nc.sync.dma_start(
                    stem_t[:, :],
                    stem_d[ts(bc, 128), ts(db, DB), :].rearrange("p d n -> p (d n)"),
                )
                out_t = otp.tile([128, DN], BF16)
                for h in range(NT):
                    col0 = h * PSW
                    ps = psD.tile([128, PSW], F32)
                    pseq = (0, 1) if h % 2 == 0 else (1, 0)
                    for pi, p in enumerate(pseq):
                        for t in range(PSW // 512):
                            c = col0 + t * 512
                            nc.tensor.matmul(
                                ps[:, ts(t, 512)],
                                lhsT=attnT[:, 2 * p:2 * p + 2, ts(bc, 128)],
                                rhs=vq[:, 2 * p:2 * p + 2, c:c + 512],
                                start=(pi == 0), stop=False,
                                perf_mode=DR,
                            )
                    # subtract SCALE*stem in PSUM: psum = SCALE*(affix-stem)
                    for t in range(PSW // 512):
                        c = col0 + t * 512
                        nc.tensor.matmul(
                            ps[:, ts(t, 512)],
                            lhsT=negI[:, :],
                            rhs=stem_t[:, c:c + 512],
                            start=False, stop=True,
                        )
                    delta_t = dlp.tile([128, PSW], BF16)
                    nc.scalar.activation(delta_t[:, :], ps[:, :], COPY,
                                         scale=1.0 / SCALE)
                    prod = prp.tile([128, PSW], BF16)
                    nc.vector.tensor_mul(
                        prod[:, :].rearrange("p (a n) -> p a n", n=N),
                        delta_t[:, :].rearrange("p (a n) -> p a n", n=N),
                        wc_sb[:, bc:bc + 1, :].to_broadcast((128, PSW // N, N)),
                    )
                    nc.vector.tensor_add(out_t[:, col0:col0 + PSW], prod[:, :],
                                         stem_t[:, col0:col0 + PSW])
                nc.sync.dma_start(
                    out_d[ts(bc, 128), ts(db, DB), :].rearrange("p d n -> p (d n)"),
                    out_t[:, :],
                )

    nc.compile()
    _CACHE[key] = nc
    return nc


def kernel(stem_form, morphosyn, pivot_logits, W_affix, affix_vocab,
           alpha, beta, phi, max_len):
    global LAST_RESULT
    stem_form = np.ascontiguousarray(np.asarray(stem_form, dtype=np.float32))
    morphosyn = np.ascontiguousarray(np.asarray(morphosyn, dtype=np.float32))
    pivot_logits = np.ascontiguousarray(np.asarray(pivot_logits, dtype=np.float32))
    W_affix = np.ascontiguousarray(np.asarray(W_affix, dtype=np.float32))
    affix_vocab = np.ascontiguousarray(np.asarray(affix_vocab, dtype=np.float32))
    abf = np.concatenate([
        np.asarray(alpha, np.float32).ravel(),
        np.asarray(beta, np.float32).ravel(),
        np.asarray(phi, np.float32).ravel(),
    ]).reshape(1, 9)

    nc = _build()

    stem_np = stem_form.astype(ml_dtypes.bfloat16)
    vocab_np = affix_vocab.astype(ml_dtypes.float8_e4m3)
    pivot_np = pivot_logits.astype(ml_dtypes.bfloat16)

    in_maps = []
    for c in range(NCORES):
        bg, dg = divmod(c, NDG)
        blo, bhi = bg * BLOC, (bg + 1) * BLOC
        dlo, dhi = dg * DLOC, (dg + 1) * DLOC
        in_maps.append({
            "stem": np.ascontiguousarray(stem_np[blo:bhi, dlo:dhi, :]),
            "vocab": np.ascontiguousarray(vocab_np[:, dlo:dhi, :]),
            "morpho": np.ascontiguousarray(morphosyn[blo:bhi]),
            "waffix": W_affix,
            "pivot": np.ascontiguousarray(pivot_np[:, :, blo:bhi, :, :]),
            "abf": abf,
        })

    LAST_RESULT = run_bass_kernel_spmd(nc, in_maps, core_ids=list(range(NCORES)))
    out = np.empty((B, D, N), dtype=np.float32)
    for c in range(NCORES):
        bg, dg = divmod(c, NDG)
        out[bg * BLOC:(bg + 1) * BLOC, dg * DLOC:(dg + 1) * DLOC, :] = \
            LAST_RESULT.results[c]["out"].astype(np.float32)
    return np.ascontiguousarray(out)


# revision 26
# speedup vs baseline: 1.4424x; 1.0502x over previous
"""Trainium2 Bass kernel for nn_MixtureCogrammar.

Computation (reference):
    attn  = softmax(morphosyn @ W_affix)                    [B, V]
    affix = attn @ affix_vocab.reshape(V, D*N)              [B, D, N]
    wC    = cumsum_n( sum_{ijk} a_i b_j f_k softmax(pivot_logits[i,j,:,k,:]) )
    out   = stem + wC * (affix - stem)

Distribution (v7): 2x4 grid — batch split 2 ways, D split 4 ways.
Per core: stem/out [512, 64, N] bf16 (16.8MB each), vocab [V, 64, N] fp8
(8.4MB), pivot for 512 batches (5.2MB bf16). wC for all 4 local batch
chunks is computed LOCALLY — no collective. (The gpsimd AllGather of the
pure D-sharded variant measured a fixed ~55us CC-handshake latency that
froze the wC-gated pipeline; the pure B-sharded variant paid 33.5MB of
replicated vocab DMA. This grid pays 8.4MB vocab and zero collectives.)

Kernel structure (from trace analysis of v2-v5):
  - fp8(e4m3) matmul in DoubleRow perf mode; attn scaled x128 on device
    and stored fp8 (measured ~263ns per 512-wide DR matmul).
  - stem subtracted IN PSUM via an identity matmul (lhsT = -128*I fp8,
    rhs = resident bf16 stem tile): psum = 128*(affix - stem), so the
    elementwise tail is only
      ScalarE: delta = psum * 1/128      (PSUM->SBUF bf16 drain)
      DVE:     prod  = delta * wC        (bf16 2x)
      DVE:     out   = prod + stem       (bf16 2x)
  - d-block outer / batch-chunk inner loop with 8-16KB per-partition DMA
    lines (DMA engines are packet-overhead limited).
  - pivot path per chunk: 20 exps with accumulate on ScalarE, weighted
    bf16 chain + scan on DVE; runs ahead of the main loop.
"""

import os
import sys

import numpy as np

for _p in ("/opt/trn_rl_repo",):
    if os.path.isdir(_p) and _p not in sys.path:
        sys.path.append(_p)

import concourse.bass as bass  # noqa: E402
import concourse.tile as tile  # noqa: E402
from concourse import bacc, mybir  # noqa: E402
from concourse.bass import ts  # noqa: E402
from concourse.bass_utils import run_bass_kernel_spmd  # noqa: E402
from concourse.masks import make_identity  # noqa: E402

import ml_dtypes  # noqa: E402

B, D, N, DM, V = 1024, 256, 256, 128, 512
NCORES = 8
NBG, NDG = 2, 4             # core grid: batch groups x d groups
BLOC = B // NBG             # 512 batches per core
NBC = BLOC // 128           # 4 local batch chunks
DLOC = D // NDG             # 64 d-values per core
NDB = 2                     # d-blocks per core
DB = DLOC // NDB            # 32 d-values per block
DN = DB * N                 # 8192 free elems per (chunk, d-block) tile
PSW = 2048                  # psum tile = 4 banks
NT = DN // PSW              # 4 h-tiles
SCALE = 128.0               # attn pre-scale so fp8 subnormals don't bite

F32 = mybir.dt.float32
F32R = mybir.dt.float32r
BF16 = mybir.dt.bfloat16
FP8 = mybir.dt.float8e4
EXP = mybir.ActivationFunctionType.Exp
COPY = mybir.ActivationFunctionType.Copy
ALU = mybir.AluOpType
DR = mybir.MatmulPerfMode.DoubleRow

LAST_RESULT = None

_CACHE = {}


def _build():
    key = 0
    if key in _CACHE:
        return _CACHE[key]

    nc = bacc.Bacc("TRN2", target_bir_lowering=False, debug=False,
                   num_devices=NCORES)

    stem_d = nc.dram_tensor("stem", [BLOC, DLOC, N], BF16, kind="ExternalInput").ap()
    vocab_d = nc.dram_tensor("vocab", [V, DLOC, N], FP8, kind="ExternalInput").ap()
    mor_d = nc.dram_tensor("morpho", [BLOC, DM], F32, kind="ExternalInput").ap()
    waff_d = nc.dram_tensor("waffix", [DM, V], F32R, kind="ExternalInput").ap()
    pv_d = nc.dram_tensor("pivot", [2, 2, BLOC, 5, N], BF16, kind="ExternalInput").ap()
    abf_d = nc.dram_tensor("abf", [1, 9], F32, kind="ExternalInput").ap()
    out_d = nc.dram_tensor("out", [BLOC, DLOC, N], BF16, kind="ExternalOutput").ap()

    from contextlib import ExitStack

    with tile.TileContext(nc) as tc, ExitStack() as ctx:
        const = ctx.enter_context(tc.tile_pool(name="const", bufs=1))

        ident = const.tile([128, 128], F32)
        make_identity(nc, ident[:, :])
        negI = const.tile([128, 128], FP8)
        nc.scalar.mul(negI[:, :], ident[:, :], -SCALE)

        attnT = const.tile([128, 4, BLOC], FP8)    # [v_part, vc, local b]
        w_bcast = const.tile([128, 20], F32)
        wsb = const.tile([128, V], F32R)           # W_affix resident
        mor_sb = const.tile([128, NBC, DM], F32)
        wc_sb = const.tile([128, NBC, N], BF16)    # local wC per chunk

        # ---------- DMAs: wC path on the Activation HWDGE queue ----------
        small = ctx.enter_context(tc.tile_pool(name="small", bufs=1))
        pvp = ctx.enter_context(tc.tile_pool(name="pv", bufs=1))
        abf = small.tile([1, 9], F32)
        nc.scalar.dma_start(abf[0:1, :], abf_d[:, :])
        pvt = []
        for bc in range(NBC):
            pvc = pvp.tile([128, 4, 5, N], BF16, tag="pvin", name=f"pv{bc}")
            for ij in range(4):
                i, j = divmod(ij, 2)
                nc.scalar.dma_start(pvc[:, ij, :, :],
                                    pv_d[i, j, ts(bc, 128), :, :])
            pvt.append(pvc)
        for bc in range(NBC):
            nc.scalar.dma_start(mor_sb[:, bc, :], mor_d[ts(bc, 128), :])
        nc.scalar.dma_start(wsb[:, :], waff_d[:, :])

        # ---------- mixture weights ----------
        eabf = small.tile([1, 9], F32)
        sums = small.tile([1, 3], F32)
        nc.scalar.activation(eabf[0:1, 0:2], abf[0:1, 0:2], EXP, accum_out=sums[0:1, 0:1])
        nc.scalar.activation(eabf[0:1, 2:4], abf[0:1, 2:4], EXP, accum_out=sums[0:1, 1:2])
        nc.scalar.activation(eabf[0:1, 4:9], abf[0:1, 4:9], EXP, accum_out=sums[0:1, 2:3])
        rsum = small.tile([1, 3], F32)
        nc.vector.reciprocal(rsum[0:1, :], sums[0:1, :])
        t4 = small.tile([1, 4], F32)
        nc.vector.tensor_mul(
            t4[0:1, :].rearrange("p (i j) -> p i j", i=2),
            eabf[0:1, 0:2].rearrange("p (i j) -> p i j", j=1).to_broadcast((1, 2, 2)),
            eabf[0:1, 2:4].rearrange("p (i j) -> p i j", i=1).to_broadcast((1, 2, 2)),
        )
        t20 = small.tile([1, 20], F32)
        nc.vector.tensor_mul(
            t20[0:1, :].rearrange("p (g k) -> p g k", g=4),
            t4[0:1, :].rearrange("p (g k) -> p g k", k=1).to_broadcast((1, 4, 5)),
            eabf[0:1, 4:9].rearrange("p (g k) -> p g k", g=1).to_broadcast((1, 4, 5)),
        )
        rr = small.tile([1, 1], F32)
        nc.vector.tensor_mul(rr[0:1, :], rsum[0:1, 0:1], rsum[0:1, 1:2])
        rrr = small.tile([1, 1], F32)
        nc.vector.tensor_mul(rrr[0:1, :], rr[0:1, :], rsum[0:1, 2:3])
        w20 = small.tile([1, 20], F32)
        nc.vector.tensor_scalar_mul(w20[0:1, :], t20[0:1, :], rrr[0:1, 0:1])
        nc.gpsimd.partition_broadcast(w_bcast[:, :], w20[0:1, :])

        # pivot softmax+mix for one chunk; emitted per-chunk, interleaved
        # with the main loop (needs no PSUM, so it runs behind the MMs)
        def pivot_chunk(bc):
            pvc = pvt[bc]
            pvE = pvp.tile([128, 20, N], BF16, tag="pvE", name=f"pvE{bc}")
            nc.scalar.activation(
                pvE[:, :, :].rearrange("p (ij k) n -> p ij k n", ij=4),
                pvc[:, :, :, :], EXP)
            # group sums: TT-add tree (2x-capable) + small reduce (the direct
            # [128,20,256] TENSOR_REDUCE measured 1x = 5.5us)
            t1 = pvp.tile([128, 20, 128], BF16, tag="t1", name=f"t1{bc}")
            nc.vector.tensor_add(t1[:, :, :], pvE[:, :, 0:128], pvE[:, :, 128:256])
            t2 = pvp.tile([128, 20, 64], BF16, tag="t2", name=f"t2{bc}")
            nc.vector.tensor_add(t2[:, :, :], t1[:, :, 0:64], t1[:, :, 64:128])
            sP = pvp.tile([128, 20, 1], F32, tag="sP", name=f"sP{bc}")
            nc.vector.reduce_sum(sP[:, :, :], t2[:, :, :],
                                 axis=mybir.AxisListType.X)
            rP = pvp.tile([128, 20], F32, tag="rP", name=f"rP{bc}")
            nc.vector.reciprocal(rP[:, :], sP[:, :, 0])
            rPw = pvp.tile([128, 20], F32, tag="rPw", name=f"rPw{bc}")
            nc.vector.tensor_mul(rPw[:, :], rP[:, :], w_bcast[:, :])
            accA = pvp.tile([128, N], BF16, tag="accA", name=f"aA{bc}")
            accB = pvp.tile([128, N], BF16, tag="accB", name=f"aB{bc}")
            nc.vector.tensor_scalar_mul(accA[:, :], pvE[:, 0, :], rPw[:, 0:1])
            cur, nxt = accA, accB
            for g in range(1, 20):
                nc.vector.scalar_tensor_tensor(
                    out=nxt[:, :], in0=pvE[:, g, :], scalar=rPw[:, g:g + 1],
                    in1=cur[:, :], op0=ALU.mult, op1=ALU.add,
                )
                cur, nxt = nxt, cur
            nc.vector.tensor_tensor_scan(
                wc_sb[:, bc, :], data0=cur[:, :], data1=cur[:, :], initial=0.0,
                op0=ALU.add, op1=ALU.bypass,
            )

        # ---------- attention (4 local chunks) ----------
        bp = ctx.enter_context(tc.tile_pool(name="attn", bufs=2))
        psB = tc.alloc_tile_pool(name="psB", bufs=2, space="PSUM")
        psT = tc.alloc_tile_pool(name="psT", bufs=1, space="PSUM")

        for bc in range(NBC):
            morT_ps = psB.tile([128, DM], F32, tag="morT_ps", name=f"mtp{bc}")
            nc.tensor.transpose(morT_ps[:, :], mor_sb[:, bc, :], ident[:, :])
            morT = bp.tile([128, DM], F32R, tag="morT", name=f"mt{bc}")
            nc.vector.tensor_copy(morT[:, :], morT_ps[:, :])
            lg_ps = psB.tile([128, V], F32, tag="lg_ps", name=f"lgp{bc}")
            nc.tensor.matmul(lg_ps[:, :], lhsT=morT[:, :], rhs=wsb[:, :],
                             start=True, stop=True)
            E = bp.tile([128, V], F32, tag="E", name=f"E{bc}")
            sE = bp.tile([128, 1], F32, tag="sE", name=f"sE{bc}")
            nc.scalar.activation(E[:, :], lg_ps[:, :], EXP, accum_out=sE[:, :])
            rE = bp.tile([128, 1], F32, tag="rE", name=f"rE{bc}")
            nc.vector.reciprocal(rE[:, :], sE[:, :])
            rEs = bp.tile([128, 1], F32, tag="rEs", name=f"rs{bc}")
            nc.vector.tensor_scalar_mul(rEs[:, :], rE[:, :], SCALE)
            attn = bp.tile([128, V], F32, tag="at", name=f"at{bc}")
            nc.vector.tensor_scalar_mul(attn[:, :], E[:, :], rEs[:, 0:1])
            tp = psT.tile([128, 4, 512], F32, tag="tp", name=f"tp{bc}")
            for vc in range(4):
                nc.tensor.transpose(tp[:, vc, 0:128], attn[:, ts(vc, 128)],
                                    ident[:, :])
            nc.scalar.copy(attnT[:, :, ts(bc, 128)], tp[:, :, 0:128])
        psT.release()
        psB.release()

        # ---------- main loop: d-block outer, batch-chunk inner ----------
        vqp = ctx.enter_context(tc.tile_pool(name="vq", bufs=1))
        stp = ctx.enter_context(tc.tile_pool(name="stem", bufs=2))
        otp = ctx.enter_context(tc.tile_pool(name="outp", bufs=2))
        dlp = ctx.enter_context(tc.tile_pool(name="delta", bufs=6))
        prp = ctx.enter_context(tc.tile_pool(name="prod", bufs=2))
        psD = ctx.enter_context(tc.tile_pool(name="psD", bufs=2, space="PSUM"))

        # full vocab resident (64KB/partition); bc-outer order spreads the
        # pivot-chain DVE work evenly across the whole kernel
        vq = vqp.tile([128, 4, NDB, DN], FP8)
        for vc in range(4):
            for db in range(NDB):
                nc.sync.dma_start(
                    vq[:, vc, db, :],
                    vocab_d[ts(vc, 128), ts(db, DB), :].rearrange("p d n -> p (d n)"),
                )
        pivot_chunk(0)
        for bc in range(NBC):
            if bc + 1 < NBC:
                pivot_chunk(bc + 1)
            for db in range(NDB):
                stem_t = stp.tile([128, DN], BF16)
                nc.sync.dma_start(
                    stem_t[:, :],
                    stem_d[ts(bc, 128), ts(db, DB), :].rearrange("p d n -> p (d n)"),
                )
                out_t = otp.tile([128, DN], BF16)
                for h in range(NT):
                    col0 = h * PSW
                    ps = psD.tile([128, PSW], F32)
                    pseq = (0, 1) if h % 2 == 0 else (1, 0)
                    for pi, p in enumerate(pseq):
                        for t in range(PSW // 512):
                            c = col0 + t * 512
                            nc.tensor.matmul(
                                ps[:, ts(t, 512)],
                                lhsT=attnT[:, 2 * p:2 * p + 2, ts(bc, 128)],
                                rhs=vq[:, 2 * p:2 * p + 2, db, c:c + 512],
                                start=(pi == 0), stop=False,
                                perf_mode=DR,
                            )
                    # subtract SCALE*stem in PSUM: psum = SCALE*(affix-stem)
                    for t in range(PSW // 512):
                        c = col0 + t * 512
                        nc.tensor.matmul(
                            ps[:, ts(t, 512)],
                            lhsT=negI[:, :],
                            rhs=stem_t[:, c:c + 512],
                            start=False, stop=True,
                        )
                    delta_t = dlp.tile([128, PSW], BF16)
                    nc.scalar.activation(delta_t[:, :], ps[:, :], COPY,
                                         scale=1.0 / SCALE)
                    prod = prp.tile([128, PSW], BF16)
                    nc.vector.tensor_mul(
                        prod[:, :].rearrange("p (a n) -> p a n", n=N),
                        delta_t[:, :].rearrange("p (a n) -> p a n", n=N),
                        wc_sb[:, bc:bc + 1, :].to_broadcast((128, PSW // N, N)),
                    )
                    nc.vector.tensor_add(out_t[:, col0:col0 + PSW], prod[:, :],
                                         stem_t[:, col0:col0 + PSW])
                nc.sync.dma_start(
                    out_d[ts(bc, 128), ts(db, DB), :].rearrange("p d n -> p (d n)"),
                    out_t[:, :],
                )

    nc.compile()
    _CACHE[key] = nc
    return nc


def kernel(stem_form, morphosyn, pivot_logits, W_affix, affix_vocab,
           alpha, beta, phi, max_len):
    global LAST_RESULT
    stem_form = np.ascontiguousarray(np.asarray(stem_form, dtype=np.float32))
    morphosyn = np.ascontiguousarray(np.asarray(morphosyn, dtype=np.float32))
    pivot_logits = np.ascontiguousarray(np.asarray(pivot_logits, dtype=np.float32))
    W_affix = np.ascontiguousarray(np.asarray(W_affix, dtype=np.float32))
    affix_vocab = np.ascontiguousarray(np.asarray(affix_vocab, dtype=np.float32))
    abf = np.concatenate([
        np.asarray(alpha, np.float32).ravel(),
        np.asarray(beta, np.float32).ravel(),
        np.asarray(phi, np.float32).ravel(),
    ]).reshape(1, 9)

    nc = _build()

    stem_np = stem_form.astype(ml_dtypes.bfloat16)
    vocab_np = affix_vocab.astype(ml_dtypes.float8_e4m3)
    pivot_np = pivot_logits.astype(ml_dtypes.bfloat16)

    in_maps = []
    for c in range(NCORES):
        bg, dg = divmod(c, NDG)
        blo, bhi = bg * BLOC, (bg + 1) * BLOC
        dlo, dhi = dg * DLOC, (dg + 1) * DLOC
        in_maps.append({
            "stem": np.ascontiguousarray(stem_np[blo:bhi, dlo:dhi, :]),
            "vocab": np.ascontiguousarray(vocab_np[:, dlo:dhi, :]),
            "morpho": np.ascontiguousarray(morphosyn[blo:bhi]),
            "waffix": W_affix,
            "pivot": np.ascontiguousarray(pivot_np[:, :, blo:bhi, :, :]),
            "abf": abf,
        })

    LAST_RESULT = run_bass_kernel_spmd(nc, in_maps, core_ids=list(range(NCORES)))
    out = np.empty((B, D, N), dtype=np.float32)
    for c in range(NCORES):
        bg, dg = divmod(c, NDG)
        out[bg * BLOC:(bg + 1) * BLOC, dg * DLOC:(dg + 1) * DLOC, :] = \
            LAST_RESULT.results[c]["out"].astype(np.float32)
    return np.ascontiguousarray(out)


# revision 28
# speedup vs baseline: 1.5665x; 1.0860x over previous
"""Trainium2 Bass kernel for nn_MixtureCogrammar.

Computation (reference):
    attn  = softmax(morphosyn @ W_affix)                    [B, V]
    affix = attn @ affix_vocab.reshape(V, D*N)              [B, D, N]
    wC    = cumsum_n( sum_{ijk} a_i b_j f_k softmax(pivot_logits[i,j,:,k,:]) )
    out   = stem + wC * (affix - stem)

Distribution: 2x4 grid — batch split 2 ways, D split 4 ways.
Per core: stem/out [512, 64, N] bf16 (16.8MB each), vocab [V, 64, N] fp8
(8.4MB), pivot for 512 batches (5.2MB bf16). wC for all 4 local batch
chunks is computed LOCALLY — no collective. (The gpsimd AllGather of the
pure D-sharded variant measured a fixed ~55us CC-handshake latency that
froze the wC-gated pipeline; the pure B-sharded variant paid 33.5MB of
replicated vocab DMA. This grid pays 8.4MB vocab and zero collectives.)

Kernel structure (from trace analysis of v2-v5):
  - fp8(e4m3) matmul in DoubleRow perf mode; attn scaled x128 on device
    and stored fp8 (measured ~263ns per 512-wide DR matmul).
  - stem subtracted IN PSUM via an identity matmul (lhsT = -128*I fp8,
    rhs = resident bf16 stem tile): psum = 128*(affix - stem), so the
    elementwise tail is only
      ScalarE: delta = psum * 1/128      (PSUM->SBUF bf16 drain)
      DVE:     prod  = delta * wC        (bf16 2x)
      DVE:     out   = prod + stem       (bf16 2x)
  - batch-chunk outer loop with the full 8.4MB fp8 vocab resident in
    SBUF (64KB/partition) and 8-16KB per-partition DMA lines (the DMA
    engines are packet-overhead limited at ~20GB/s each).
  - pivot path per chunk: one fused exp on ScalarE, TT-add tree + small
    reduce for the group sums, weighted STT chain + scan on DVE; emitted
    one chunk ahead of the main-loop segment that consumes it, so the
    DVE-heavy chain work spreads across the whole kernel.
"""

import os
import sys

import numpy as np

for _p in ("/opt/trn_rl_repo",):
    if os.path.isdir(_p) and _p not in sys.path:
        sys.path.append(_p)

import concourse.bass as bass  # noqa: E402
import concourse.tile as tile  # noqa: E402
from concourse import bacc, mybir  # noqa: E402
from concourse.bass import ts  # noqa: E402
from concourse.bass_utils import run_bass_kernel_spmd  # noqa: E402
from concourse.masks import make_identity  # noqa: E402

import ml_dtypes  # noqa: E402

B, D, N, DM, V = 1024, 256, 256, 128, 512
NCORES = 8
NBG, NDG = 2, 4             # core grid: batch groups x d groups
BLOC = B // NBG             # 512 batches per core
NBC = BLOC // 128           # 4 local batch chunks
DLOC = D // NDG             # 64 d-values per core
NDB = 2                     # d-blocks per core
DB = DLOC // NDB            # 32 d-values per block
DN = DB * N                 # 8192 free elems per (chunk, d-block) tile
PSW = 2048                  # psum tile = 4 banks
NT = DN // PSW              # 4 h-tiles
SCALE = 128.0               # attn pre-scale so fp8 subnormals don't bite

F32 = mybir.dt.float32
F32R = mybir.dt.float32r
BF16 = mybir.dt.bfloat16
FP8 = mybir.dt.float8e4
EXP = mybir.ActivationFunctionType.Exp
COPY = mybir.ActivationFunctionType.Copy
ALU = mybir.AluOpType
DR = mybir.MatmulPerfMode.DoubleRow

LAST_RESULT = None

_CACHE = {}


def _build():
    key = 0
    if key in _CACHE:
        return _CACHE[key]

    nc = bacc.Bacc("TRN2", target_bir_lowering=False, debug=False,
                   num_devices=NCORES)

    stem_d = nc.dram_tensor("stem", [BLOC, DLOC, N], BF16, kind="ExternalInput").ap()
    vocab_d = nc.dram_tensor("vocab", [V, DLOC, N], FP8, kind="ExternalInput").ap()
    mor_d = nc.dram_tensor("morpho", [BLOC, DM], F32, kind="ExternalInput").ap()
    waff_d = nc.dram_tensor("waffix", [DM, V], F32R, kind="ExternalInput").ap()
    pv_d = nc.dram_tensor("pivot", [2, 2, BLOC, 5, N], BF16, kind="ExternalInput").ap()
    abf_d = nc.dram_tensor("abf", [1, 9], F32, kind="ExternalInput").ap()
    out_d = nc.dram_tensor("out", [BLOC, DLOC, N], BF16, kind="ExternalOutput").ap()

    from contextlib import ExitStack

    with tile.TileContext(nc) as tc, ExitStack() as ctx:
        const = ctx.enter_context(tc.tile_pool(name="const", bufs=1))

        ident = const.tile([128, 128], F32)
        make_identity(nc, ident[:, :])
        negI = const.tile([128, 128], FP8)
        nc.scalar.mul(negI[:, :], ident[:, :], -SCALE)

        attnT = const.tile([128, 4, BLOC], FP8)    # [v_part, vc, local b]
        w_bcast = const.tile([128, 20], F32)
        wsb = const.tile([128, V], F32R)           # W_affix resident
        mor_sb = const.tile([128, NBC, DM], F32)
        wc_sb = const.tile([128, NBC, N], BF16)    # local wC per chunk

        # ---------- DMAs: wC path on the Activation HWDGE queue ----------
        small = ctx.enter_context(tc.tile_pool(name="small", bufs=1))
        pvp = ctx.enter_context(tc.tile_pool(name="pv", bufs=1))
        abf = small.tile([1, 9], F32)
        nc.scalar.dma_start(abf[0:1, :], abf_d[:, :])
        pvt = []
        for bc in range(NBC):
            pvc = pvp.tile([128, 4, 5, N], BF16, tag="pvin", name=f"pv{bc}")
            for ij in range(4):
                i, j = divmod(ij, 2)
                nc.scalar.dma_start(pvc[:, ij, :, :],
                                    pv_d[i, j, ts(bc, 128), :, :])
            pvt.append(pvc)
        for bc in range(NBC):
            nc.scalar.dma_start(mor_sb[:, bc, :], mor_d[ts(bc, 128), :])
        nc.scalar.dma_start(wsb[:, :], waff_d[:, :])

        # ---------- mixture weights ----------
        eabf = small.tile([1, 9], F32)
        sums = small.tile([1, 3], F32)
        nc.scalar.activation(eabf[0:1, 0:2], abf[0:1, 0:2], EXP, accum_out=sums[0:1, 0:1])
        nc.scalar.activation(eabf[0:1, 2:4], abf[0:1, 2:4], EXP, accum_out=sums[0:1, 1:2])
        nc.scalar.activation(eabf[0:1, 4:9], abf[0:1, 4:9], EXP, accum_out=sums[0:1, 2:3])
        rsum = small.tile([1, 3], F32)
        nc.vector.reciprocal(rsum[0:1, :], sums[0:1, :])
        t4 = small.tile([1, 4], F32)
        nc.vector.tensor_mul(
            t4[0:1, :].rearrange("p (i j) -> p i j", i=2),
            eabf[0:1, 0:2].rearrange("p (i j) -> p i j", j=1).to_broadcast((1, 2, 2)),
            eabf[0:1, 2:4].rearrange("p (i j) -> p i j", i=1).to_broadcast((1, 2, 2)),
        )
        t20 = small.tile([1, 20], F32)
        nc.vector.tensor_mul(
            t20[0:1, :].rearrange("p (g k) -> p g k", g=4),
            t4[0:1, :].rearrange("p (g k) -> p g k", k=1).to_broadcast((1, 4, 5)),
            eabf[0:1, 4:9].rearrange("p (g k) -> p g k", g=1).to_broadcast((1, 4, 5)),
        )
        rr = small.tile([1, 1], F32)
        nc.vector.tensor_mul(rr[0:1, :], rsum[0:1, 0:1], rsum[0:1, 1:2])
        rrr = small.tile([1, 1], F32)
        nc.vector.tensor_mul(rrr[0:1, :], rr[0:1, :], rsum[0:1, 2:3])
        w20 = small.tile([1, 20], F32)
        nc.vector.tensor_scalar_mul(w20[0:1, :], t20[0:1, :], rrr[0:1, 0:1])
        nc.gpsimd.partition_broadcast(w_bcast[:, :], w20[0:1, :])

        # pivot softmax+mix for one chunk; emitted per-chunk, interleaved
        # with the main loop (needs no PSUM, so it runs behind the MMs)
        def pivot_chunk(bc):
            pvc = pvt[bc]
            pvE = pvp.tile([128, 20, N], BF16, tag="pvE", name=f"pvE{bc}")
            nc.scalar.activation(
                pvE[:, :, :].rearrange("p (ij k) n -> p ij k n", ij=4),
                pvc[:, :, :, :], EXP)
            # group sums: TT-add tree (2x-capable) + small reduce (the direct
            # [128,20,256] TENSOR_REDUCE measured 1x = 5.5us)
            t1 = pvp.tile([128, 20, 128], BF16, tag="t1", name=f"t1{bc}")
            nc.vector.tensor_add(t1[:, :, :], pvE[:, :, 0:128], pvE[:, :, 128:256])
            t2 = pvp.tile([128, 20, 64], BF16, tag="t2", name=f"t2{bc}")
            nc.vector.tensor_add(t2[:, :, :], t1[:, :, 0:64], t1[:, :, 64:128])
            sP = pvp.tile([128, 20, 1], F32, tag="sP", name=f"sP{bc}")
            nc.vector.reduce_sum(sP[:, :, :], t2[:, :, :],
                                 axis=mybir.AxisListType.X)
            rP = pvp.tile([128, 20], F32, tag="rP", name=f"rP{bc}")
            nc.vector.reciprocal(rP[:, :], sP[:, :, 0])
            rPw = pvp.tile([128, 20], F32, tag="rPw", name=f"rPw{bc}")
            nc.vector.tensor_mul(rPw[:, :], rP[:, :], w_bcast[:, :])
            accA = pvp.tile([128, N], BF16, tag="accA", name=f"aA{bc}")
            accB = pvp.tile([128, N], BF16, tag="accB", name=f"aB{bc}")
            nc.vector.tensor_scalar_mul(accA[:, :], pvE[:, 0, :], rPw[:, 0:1])
            cur, nxt = accA, accB
            for g in range(1, 20):
                nc.vector.scalar_tensor_tensor(
                    out=nxt[:, :], in0=pvE[:, g, :], scalar=rPw[:, g:g + 1],
                    in1=cur[:, :], op0=ALU.mult, op1=ALU.add,
                )
                cur, nxt = nxt, cur
            nc.vector.tensor_tensor_scan(
                wc_sb[:, bc, :], data0=cur[:, :], data1=cur[:, :], initial=0.0,
                op0=ALU.add, op1=ALU.bypass,
            )

        # ---------- attention (4 local chunks) ----------
        bp = ctx.enter_context(tc.tile_pool(name="attn", bufs=2))
        psB = tc.alloc_tile_pool(name="psB", bufs=2, space="PSUM")
        psT = tc.alloc_tile_pool(name="psT", bufs=1, space="PSUM")

        for bc in range(NBC):
            morT_ps = psB.tile([128, DM], F32, tag="morT_ps", name=f"mtp{bc}")
            nc.tensor.transpose(morT_ps[:, :], mor_sb[:, bc, :], ident[:, :])
            morT = bp.tile([128, DM], F32R, tag="morT", name=f"mt{bc}")
            nc.vector.tensor_copy(morT[:, :], morT_ps[:, :])
            lg_ps = psB.tile([128, V], F32, tag="lg_ps", name=f"lgp{bc}")
            nc.tensor.matmul(lg_ps[:, :], lhsT=morT[:, :], rhs=wsb[:, :],
                             start=True, stop=True)
            E = bp.tile([128, V], F32, tag="E", name=f"E{bc}")
            sE = bp.tile([128, 1], F32, tag="sE", name=f"sE{bc}")
            nc.scalar.activation(E[:, :], lg_ps[:, :], EXP, accum_out=sE[:, :])
            rE = bp.tile([128, 1], F32, tag="rE", name=f"rE{bc}")
            nc.vector.reciprocal(rE[:, :], sE[:, :])
            rEs = bp.tile([128, 1], F32, tag="rEs", name=f"rs{bc}")
            nc.vector.tensor_scalar_mul(rEs[:, :], rE[:, :], SCALE)
            attn = bp.tile([128, V], F32, tag="at", name=f"at{bc}")
            nc.vector.tensor_scalar_mul(attn[:, :], E[:, :], rEs[:, 0:1])
            tp = psT.tile([128, 4, 512], F32, tag="tp", name=f"tp{bc}")
            for vc in range(4):
                nc.tensor.transpose(tp[:, vc, 0:128], attn[:, ts(vc, 128)],
                                    ident[:, :])
            nc.scalar.copy(attnT[:, :, ts(bc, 128)], tp[:, :, 0:128])
        psT.release()
        psB.release()

        # ---------- main loop: d-block outer, batch-chunk inner ----------
        vqp = ctx.enter_context(tc.tile_pool(name="vq", bufs=1))
        stp = ctx.enter_context(tc.tile_pool(name="stem", bufs=2))
        otp = ctx.enter_context(tc.tile_pool(name="outp", bufs=2))
        dlp = ctx.enter_context(tc.tile_pool(name="delta", bufs=6))
        prp = ctx.enter_context(tc.tile_pool(name="prod", bufs=2))
        psD = ctx.enter_context(tc.tile_pool(name="psD", bufs=2, space="PSUM"))

        # full vocab resident (64KB/partition); bc-outer order spreads the
        # pivot-chain DVE work evenly across the whole kernel
        vq = vqp.tile([128, 4, NDB, DN], FP8)
        for vc in range(4):
            for db in range(NDB):
                nc.sync.dma_start(
                    vq[:, vc, db, :],
                    vocab_d[ts(vc, 128), ts(db, DB), :].rearrange("p d n -> p (d n)"),
                )
        pivot_chunk(0)
        for bc in range(NBC):
            for db in range(NDB):
                if db == 1 and bc + 1 < NBC:
                    pivot_chunk(bc + 1)
                stem_t = stp.tile([128, DN], BF16)
                for q in range(NT):
                    nc.sync.dma_start(
                        stem_t[:, ts(q, PSW)],
                        stem_d[ts(bc, 128),
                               bass.ds(db * DB + q * (PSW // N), PSW // N), :]
                        .rearrange("p d n -> p (d n)"),
                    )
                out_t = otp.tile([128, DN], BF16)
                for h in range(NT):
                    col0 = h * PSW
                    ps = psD.tile([128, PSW], F32)
                    pseq = (0, 1) if h % 2 == 0 else (1, 0)
                    for pi, p in enumerate(pseq):
                        for t in range(PSW // 512):
                            c = col0 + t * 512
                            nc.tensor.matmul(
                                ps[:, ts(t, 512)],
                                lhsT=attnT[:, 2 * p:2 * p + 2, ts(bc, 128)],
                                rhs=vq[:, 2 * p:2 * p + 2, db, c:c + 512],
                                start=(pi == 0), stop=False,
                                perf_mode=DR,
                            )
                    # subtract SCALE*stem in PSUM: psum = SCALE*(affix-stem)
                    for t in range(PSW // 512):
                        c = col0 + t * 512
                        nc.tensor.matmul(
                            ps[:, ts(t, 512)],
                            lhsT=negI[:, :],
                            rhs=stem_t[:, c:c + 512],
                            start=False, stop=True,
                        )
                    delta_t = dlp.tile([128, PSW], BF16)
                    nc.scalar.activation(delta_t[:, :], ps[:, :], COPY,
                                         scale=1.0 / SCALE)
                    prod = prp.tile([128, PSW], BF16)
                    nc.vector.tensor_mul(
                        prod[:, :].rearrange("p (a n) -> p a n", n=N),
                        delta_t[:, :].rearrange("p (a n) -> p a n", n=N),
                        wc_sb[:, bc:bc + 1, :].to_broadcast((128, PSW // N, N)),
                    )
                    nc.vector.tensor_add(out_t[:, col0:col0 + PSW], prod[:, :],
                                         stem_t[:, col0:col0 + PSW])
                nc.sync.dma_start(
                    out_d[ts(bc, 128), ts(db, DB), :].rearrange("p d n -> p (d n)"),
                    out_t[:, :],
                )

    nc.compile()
    _CACHE[key] = nc
    return nc


def kernel(stem_form, morphosyn, pivot_logits, W_affix, affix_vocab,
           alpha, beta, phi, max_len):
    global LAST_RESULT
    stem_form = np.ascontiguousarray(np.asarray(stem_form, dtype=np.float32))
    morphosyn = np.ascontiguousarray(np.asarray(morphosyn, dtype=np.float32))
    pivot_logits = np.ascontiguousarray(np.asarray(pivot_logits, dtype=np.float32))
    W_affix = np.ascontiguousarray(np.asarray(W_affix, dtype=np.float32))
    affix_vocab = np.ascontiguousarray(np.asarray(affix_vocab, dtype=np.float32))
    abf = np.concatenate([
        np.asarray(alpha, np.float32).ravel(),
        np.asarray(beta, np.float32).ravel(),
        np.asarray(phi, np.float32).ravel(),
    ]).reshape(1, 9)

    nc = _build()

    stem_np = stem_form.astype(ml_dtypes.bfloat16)
    vocab_np = affix_vocab.astype(ml_dtypes.float8_e4m3)
    pivot_np = pivot_logits.astype(ml_dtypes.bfloat16)

    in_maps = []
    for c in range(NCORES):
        bg, dg = divmod(c, NDG)
        blo, bhi = bg * BLOC, (bg + 1) * BLOC
        dlo, dhi = dg * DLOC, (dg + 1) * DLOC
        in_maps.append({
            "stem": np.ascontiguousarray(stem_np[blo:bhi, dlo:dhi, :]),
            "vocab": np.ascontiguousarray(vocab_np[:, dlo:dhi, :]),
            "morpho": np.ascontiguousarray(morphosyn[blo:bhi]),
            "waffix": W_affix,
            "pivot": np.ascontiguousarray(pivot_np[:, :, blo:bhi, :, :]),
            "abf": abf,
        })

    LAST_RESULT = run_bass_kernel_spmd(nc, in_maps, core_ids=list(range(NCORES)))
    out = np.empty((B, D, N), dtype=np.float32)
    for c in range(NCORES):
        bg, dg = divmod(c, NDG)
        out[bg * BLOC:(bg + 1) * BLOC, dg * DLOC:(dg + 1) * DLOC, :] = \
            LAST_RESULT.results[c]["out"].astype(np.float32)
    return np.ascontiguousarray(out)


# revision 29
# speedup vs baseline: 1.5985x; 1.0204x over previous
"""Trainium2 Bass kernel for nn_MixtureCogrammar.

Computation (reference):
    attn  = softmax(morphosyn @ W_affix)                    [B, V]
    affix = attn @ affix_vocab.reshape(V, D*N)              [B, D, N]
    wC    = cumsum_n( sum_{ijk} a_i b_j f_k softmax(pivot_logits[i,j,:,k,:]) )
    out   = stem + wC * (affix - stem)

Distribution: 2x4 grid — batch split 2 ways, D split 4 ways.
Per core: stem/out [512, 64, N] bf16 (16.8MB each), vocab [V, 64, N] fp8
(8.4MB), pivot for 512 batches (5.2MB bf16). wC for all 4 local batch
chunks is computed LOCALLY — no collective. (The gpsimd AllGather of the
pure D-sharded variant measured a fixed ~55us CC-handshake latency that
froze the wC-gated pipeline; the pure B-sharded variant paid 33.5MB of
replicated vocab DMA. This grid pays 8.4MB vocab and zero collectives.)

Kernel structure (from trace analysis of v2-v5):
  - fp8(e4m3) matmul in DoubleRow perf mode; attn scaled x128 on device
    and stored fp8 (measured ~263ns per 512-wide DR matmul).
  - stem subtracted IN PSUM via an identity matmul (lhsT = -128*I fp8,
    rhs = resident bf16 stem tile): psum = 128*(affix - stem), so the
    elementwise tail is only
      ScalarE: delta = psum * 1/128      (PSUM->SBUF bf16 drain)
      DVE:     prod  = delta * wC        (bf16 2x)
      DVE:     out   = prod + stem       (bf16 2x)
  - batch-chunk outer loop with the full 8.4MB fp8 vocab resident in
    SBUF (64KB/partition) and 8-16KB per-partition DMA lines (the DMA
    engines are packet-overhead limited at ~20GB/s each).
  - pivot path per chunk: one fused exp on ScalarE, TT-add tree + small
    reduce for the group sums, weighted STT chain + scan on DVE; emitted
    one chunk ahead of the main-loop segment that consumes it, so the
    DVE-heavy chain work spreads across the whole kernel.
"""

import os
import sys

import numpy as np

for _p in ("/opt/trn_rl_repo",):
    if os.path.isdir(_p) and _p not in sys.path:
        sys.path.append(_p)

import concourse.bass as bass  # noqa: E402
import concourse.tile as tile  # noqa: E402
from concourse import bacc, mybir  # noqa: E402
from concourse.bass import ts  # noqa: E402
from concourse.bass_utils import run_bass_kernel_spmd  # noqa: E402
from concourse.masks import make_identity  # noqa: E402

import ml_dtypes  # noqa: E402

B, D, N, DM, V = 1024, 256, 256, 128, 512
NCORES = 8
NBG, NDG = 2, 4             # core grid: batch groups x d groups
BLOC = B // NBG             # 512 batches per core
NBC = BLOC // 128           # 4 local batch chunks
DLOC = D // NDG             # 64 d-values per core
NDB = 2                     # d-blocks per core
DB = DLOC // NDB            # 32 d-values per block
DN = DB * N                 # 8192 free elems per (chunk, d-block) tile
PSW = 2048                  # psum tile = 4 banks
NT = DN // PSW              # 4 h-tiles
SCALE = 128.0               # attn pre-scale so fp8 subnormals don't bite

F32 = mybir.dt.float32
F32R = mybir.dt.float32r
BF16 = mybir.dt.bfloat16
FP8 = mybir.dt.float8e4
EXP = mybir.ActivationFunctionType.Exp
COPY = mybir.ActivationFunctionType.Copy
ALU = mybir.AluOpType
DR = mybir.MatmulPerfMode.DoubleRow

LAST_RESULT = None

_CACHE = {}


def _build():
    key = 0
    if key in _CACHE:
        return _CACHE[key]

    nc = bacc.Bacc("TRN2", target_bir_lowering=False, debug=False,
                   num_devices=NCORES)

    stem_d = nc.dram_tensor("stem", [BLOC, DLOC, N], BF16, kind="ExternalInput").ap()
    vocab_d = nc.dram_tensor("vocab", [V, DLOC, N], FP8, kind="ExternalInput").ap()
    mor_d = nc.dram_tensor("morpho", [BLOC, DM], F32, kind="ExternalInput").ap()
    waff_d = nc.dram_tensor("waffix", [DM, V], F32R, kind="ExternalInput").ap()
    pv_d = nc.dram_tensor("pivot", [2, 2, BLOC, 5, N], BF16, kind="ExternalInput").ap()
    abf_d = nc.dram_tensor("abf", [1, 9], F32, kind="ExternalInput").ap()
    out_d = nc.dram_tensor("out", [BLOC, DLOC, N], BF16, kind="ExternalOutput").ap()

    from contextlib import ExitStack

    with tile.TileContext(nc) as tc, ExitStack() as ctx:
        const = ctx.enter_context(tc.tile_pool(name="const", bufs=1))

        ident = const.tile([128, 128], F32)
        make_identity(nc, ident[:, :])
        negI = const.tile([128, 128], FP8)
        nc.scalar.mul(negI[:, :], ident[:, :], -SCALE)

        attnT = const.tile([128, 4, BLOC], FP8)    # [v_part, vc, local b]
        w_bcast = const.tile([128, 20], F32)
        wsb = const.tile([128, V], F32R)           # W_affix resident
        mor_sb = const.tile([128, NBC, DM], F32)
        wc_sb = const.tile([128, NBC, N], BF16)    # local wC per chunk

        # ---------- DMAs: wC path on the Activation HWDGE queue ----------
        small = ctx.enter_context(tc.tile_pool(name="small", bufs=1))
        pvp = ctx.enter_context(tc.tile_pool(name="pv", bufs=1))
        abf = small.tile([1, 9], F32)
        nc.scalar.dma_start(abf[0:1, :], abf_d[:, :])
        pvt = []
        for bc in range(NBC):
            pvc = pvp.tile([128, 4, 5, N], BF16, tag="pvin", name=f"pv{bc}")
            for ij in range(4):
                i, j = divmod(ij, 2)
                nc.scalar.dma_start(pvc[:, ij, :, :],
                                    pv_d[i, j, ts(bc, 128), :, :])
            pvt.append(pvc)
        for bc in range(NBC):
            nc.scalar.dma_start(mor_sb[:, bc, :], mor_d[ts(bc, 128), :])
        nc.scalar.dma_start(wsb[:, :], waff_d[:, :])

        # ---------- mixture weights ----------
        eabf = small.tile([1, 9], F32)
        sums = small.tile([1, 3], F32)
        nc.scalar.activation(eabf[0:1, 0:2], abf[0:1, 0:2], EXP, accum_out=sums[0:1, 0:1])
        nc.scalar.activation(eabf[0:1, 2:4], abf[0:1, 2:4], EXP, accum_out=sums[0:1, 1:2])
        nc.scalar.activation(eabf[0:1, 4:9], abf[0:1, 4:9], EXP, accum_out=sums[0:1, 2:3])
        rsum = small.tile([1, 3], F32)
        nc.vector.reciprocal(rsum[0:1, :], sums[0:1, :])
        t4 = small.tile([1, 4], F32)
        nc.vector.tensor_mul(
            t4[0:1, :].rearrange("p (i j) -> p i j", i=2),
            eabf[0:1, 0:2].rearrange("p (i j) -> p i j", j=1).to_broadcast((1, 2, 2)),
            eabf[0:1, 2:4].rearrange("p (i j) -> p i j", i=1).to_broadcast((1, 2, 2)),
        )
        t20 = small.tile([1, 20], F32)
        nc.vector.tensor_mul(
            t20[0:1, :].rearrange("p (g k) -> p g k", g=4),
            t4[0:1, :].rearrange("p (g k) -> p g k", k=1).to_broadcast((1, 4, 5)),
            eabf[0:1, 4:9].rearrange("p (g k) -> p g k", g=1).to_broadcast((1, 4, 5)),
        )
        rr = small.tile([1, 1], F32)
        nc.vector.tensor_mul(rr[0:1, :], rsum[0:1, 0:1], rsum[0:1, 1:2])
        rrr = small.tile([1, 1], F32)
        nc.vector.tensor_mul(rrr[0:1, :], rr[0:1, :], rsum[0:1, 2:3])
        w20 = small.tile([1, 20], F32)
        nc.vector.tensor_scalar_mul(w20[0:1, :], t20[0:1, :], rrr[0:1, 0:1])
        nc.gpsimd.partition_broadcast(w_bcast[:, :], w20[0:1, :])

        # pivot softmax+mix for one chunk; emitted per-chunk, interleaved
        # with the main loop (needs no PSUM, so it runs behind the MMs)
        def pivot_chunk(bc):
            pvc = pvt[bc]
            pvE = pvp.tile([128, 20, N], BF16, tag="pvE", name=f"pvE{bc}")
            nc.scalar.activation(
                pvE[:, :, :].rearrange("p (ij k) n -> p ij k n", ij=4),
                pvc[:, :, :, :], EXP)
            # group sums: TT-add tree (2x-capable) + small reduce (the direct
            # [128,20,256] TENSOR_REDUCE measured 1x = 5.5us)
            t1 = pvp.tile([128, 20, 128], BF16, tag="t1", name=f"t1{bc}")
            nc.vector.tensor_add(t1[:, :, :], pvE[:, :, 0:128], pvE[:, :, 128:256])
            t2 = pvp.tile([128, 20, 64], BF16, tag="t2", name=f"t2{bc}")
            nc.vector.tensor_add(t2[:, :, :], t1[:, :, 0:64], t1[:, :, 64:128])
            sP = pvp.tile([128, 20, 1], F32, tag="sP", name=f"sP{bc}")
            nc.vector.reduce_sum(sP[:, :, :], t2[:, :, :],
                                 axis=mybir.AxisListType.X)
            rP = pvp.tile([128, 20], F32, tag="rP", name=f"rP{bc}")
            nc.vector.reciprocal(rP[:, :], sP[:, :, 0])
            rPw = pvp.tile([128, 20], F32, tag="rPw", name=f"rPw{bc}")
            nc.vector.tensor_mul(rPw[:, :], rP[:, :], w_bcast[:, :])
            accA = pvp.tile([128, N], BF16, tag="accA", name=f"aA{bc}")
            accB = pvp.tile([128, N], BF16, tag="accB", name=f"aB{bc}")
            nc.vector.tensor_scalar_mul(accA[:, :], pvE[:, 0, :], rPw[:, 0:1])
            cur, nxt = accA, accB
            for g in range(1, 20):
                nc.vector.scalar_tensor_tensor(
                    out=nxt[:, :], in0=pvE[:, g, :], scalar=rPw[:, g:g + 1],
                    in1=cur[:, :], op0=ALU.mult, op1=ALU.add,
                )
                cur, nxt = nxt, cur
            nc.vector.tensor_tensor_scan(
                wc_sb[:, bc, :], data0=cur[:, :], data1=cur[:, :], initial=0.0,
                op0=ALU.add, op1=ALU.bypass,
            )

        # ---------- attention (4 local chunks) ----------
        bp = ctx.enter_context(tc.tile_pool(name="attn", bufs=2))
        psB = tc.alloc_tile_pool(name="psB", bufs=2, space="PSUM")
        psT = tc.alloc_tile_pool(name="psT", bufs=1, space="PSUM")

        for bc in range(NBC):
            morT_ps = psB.tile([128, DM], F32, tag="morT_ps", name=f"mtp{bc}")
            nc.tensor.transpose(morT_ps[:, :], mor_sb[:, bc, :], ident[:, :])
            morT = bp.tile([128, DM], F32R, tag="morT", name=f"mt{bc}")
            nc.vector.tensor_copy(morT[:, :], morT_ps[:, :])
            lg_ps = psB.tile([128, V], F32, tag="lg_ps", name=f"lgp{bc}")
            nc.tensor.matmul(lg_ps[:, :], lhsT=morT[:, :], rhs=wsb[:, :],
                             start=True, stop=True)
            E = bp.tile([128, V], F32, tag="E", name=f"E{bc}")
            sE = bp.tile([128, 1], F32, tag="sE", name=f"sE{bc}")
            nc.scalar.activation(E[:, :], lg_ps[:, :], EXP, accum_out=sE[:, :])
            rE = bp.tile([128, 1], F32, tag="rE", name=f"rE{bc}")
            nc.vector.reciprocal(rE[:, :], sE[:, :])
            rEs = bp.tile([128, 1], F32, tag="rEs", name=f"rs{bc}")
            nc.vector.tensor_scalar_mul(rEs[:, :], rE[:, :], SCALE)
            attn = bp.tile([128, V], F32, tag="at", name=f"at{bc}")
            nc.vector.tensor_scalar_mul(attn[:, :], E[:, :], rEs[:, 0:1])
            tp = psT.tile([128, 4, 512], F32, tag="tp", name=f"tp{bc}")
            for vc in range(4):
                nc.tensor.transpose(tp[:, vc, 0:128], attn[:, ts(vc, 128)],
                                    ident[:, :])
            nc.scalar.copy(attnT[:, :, ts(bc, 128)], tp[:, :, 0:128])
        psT.release()
        psB.release()

        # ---------- main loop: d-block outer, batch-chunk inner ----------
        vqp = ctx.enter_context(tc.tile_pool(name="vq", bufs=1))
        stp = ctx.enter_context(tc.tile_pool(name="stem", bufs=2))
        otp = ctx.enter_context(tc.tile_pool(name="outp", bufs=2))
        dlp = ctx.enter_context(tc.tile_pool(name="delta", bufs=6))
        prp = ctx.enter_context(tc.tile_pool(name="prod", bufs=2))
        psD = ctx.enter_context(tc.tile_pool(name="psD", bufs=2, space="PSUM"))

        # full vocab resident (64KB/partition); bc-outer order spreads the
        # pivot-chain DVE work evenly across the whole kernel
        vq = vqp.tile([128, 4, NDB, DN], FP8)
        for vc in range(4):
            for db in range(NDB):
                nc.sync.dma_start(
                    vq[:, vc, db, :],
                    vocab_d[ts(vc, 128), ts(db, DB), :].rearrange("p d n -> p (d n)"),
                )
        pivot_chunk(0)
        for bc in range(NBC):
            for db in range(NDB):
                if db == 1 and bc + 1 < NBC:
                    pivot_chunk(bc + 1)
                stem_t = stp.tile([128, DN], BF16)
                for q in range(NT):
                    nc.sync.dma_start(
                        stem_t[:, ts(q, PSW)],
                        stem_d[ts(bc, 128),
                               bass.ds(db * DB + q * (PSW // N), PSW // N), :]
                        .rearrange("p d n -> p (d n)"),
                    )
                out_t = otp.tile([128, DN], BF16)
                for h in range(NT):
                    col0 = h * PSW
                    ps = psD.tile([128, PSW], F32)
                    pseq = (0, 1) if h % 2 == 0 else (1, 0)
                    for pi, p in enumerate(pseq):
                        for t in range(PSW // 512):
                            c = col0 + t * 512
                            nc.tensor.matmul(
                                ps[:, ts(t, 512)],
                                lhsT=attnT[:, 2 * p:2 * p + 2, ts(bc, 128)],
                                rhs=vq[:, 2 * p:2 * p + 2, db, c:c + 512],
                                start=(pi == 0), stop=False,
                                perf_mode=DR,
                            )
                    # subtract SCALE*stem in PSUM: psum = SCALE*(affix-stem)
                    for t in range(PSW // 512):
                        c = col0 + t * 512
                        nc.tensor.matmul(
                            ps[:, ts(t, 512)],
                            lhsT=negI[:, :],
                            rhs=stem_t[:, c:c + 512],
                            start=False, stop=True,
                        )
                    delta_t = dlp.tile([128, PSW], BF16)
                    nc.scalar.activation(delta_t[:, :], ps[:, :], COPY,
                                         scale=1.0 / SCALE)
                    prod = prp.tile([128, PSW], BF16)
                    nc.vector.tensor_mul(
                        prod[:, :].rearrange("p (a n) -> p a n", n=N),
                        delta_t[:, :].rearrange("p (a n) -> p a n", n=N),
                        wc_sb[:, bc:bc + 1, :].to_broadcast((128, PSW // N, N)),
                    )
                    nc.vector.tensor_add(out_t[:, col0:col0 + PSW], prod[:, :],
                                         stem_t[:, col0:col0 + PSW])
                for q in range(2):
                    nc.sync.dma_start(
                        out_d[ts(bc, 128),
                              bass.ds(db * DB + q * (DB // 2), DB // 2), :]
                        .rearrange("p d n -> p (d n)"),
                        out_t[:, ts(q, DN // 2)],
                    )

    nc.compile()
    _CACHE[key] = nc
    return nc


def kernel(stem_form, morphosyn, pivot_logits, W_affix, affix_vocab,
           alpha, beta, phi, max_len):
    global LAST_RESULT
    stem_form = np.ascontiguousarray(np.asarray(stem_form, dtype=np.float32))
    morphosyn = np.ascontiguousarray(np.asarray(morphosyn, dtype=np.float32))
    pivot_logits = np.ascontiguousarray(np.asarray(pivot_logits, dtype=np.float32))
    W_affix = np.ascontiguousarray(np.asarray(W_affix, dtype=np.float32))
    affix_vocab = np.ascontiguousarray(np.asarray(affix_vocab, dtype=np.float32))
    abf = np.concatenate([
        np.asarray(alpha, np.float32).ravel(),
        np.asarray(beta, np.float32).ravel(),
        np.asarray(phi, np.float32).ravel(),
    ]).reshape(1, 9)

    nc = _build()

    stem_np = stem_form.astype(ml_dtypes.bfloat16)
    vocab_np = affix_vocab.astype(ml_dtypes.float8_e4m3)
    pivot_np = pivot_logits.astype(ml_dtypes.bfloat16)

    in_maps = []
    for c in range(NCORES):
        bg, dg = divmod(c, NDG)
        blo, bhi = bg * BLOC, (bg + 1) * BLOC
        dlo, dhi = dg * DLOC, (dg + 1) * DLOC
        in_maps.append({
            "stem": np.ascontiguousarray(stem_np[blo:bhi, dlo:dhi, :]),
            "vocab": np.ascontiguousarray(vocab_np[:, dlo:dhi, :]),
            "morpho": np.ascontiguousarray(morphosyn[blo:bhi]),
            "waffix": W_affix,
            "pivot": np.ascontiguousarray(pivot_np[:, :, blo:bhi, :, :]),
            "abf": abf,
        })

    LAST_RESULT = run_bass_kernel_spmd(nc, in_maps, core_ids=list(range(NCORES)))
    out = np.empty((B, D, N), dtype=np.float32)
    for c in range(NCORES):
        bg, dg = divmod(c, NDG)
        out[bg * BLOC:(bg + 1) * BLOC, dg * DLOC:(dg + 1) * DLOC, :] = \
            LAST_RESULT.results[c]["out"].astype(np.float32)
    return np.ascontiguousarray(out)


# revision 30
# speedup vs baseline: 1.6133x; 1.0092x over previous
"""Trainium2 Bass kernel for nn_MixtureCogrammar.

Computation (reference):
    attn  = softmax(morphosyn @ W_affix)                    [B, V]
    affix = attn @ affix_vocab.reshape(V, D*N)              [B, D, N]
    wC    = cumsum_n( sum_{ijk} a_i b_j f_k softmax(pivot_logits[i,j,:,k,:]) )
    out   = stem + wC * (affix - stem)

Distribution: 2x4 grid — batch split 2 ways, D split 4 ways.
Per core: stem/out [512, 64, N] bf16 (16.8MB each), vocab [V, 64, N] fp8
(8.4MB), pivot for 512 batches (5.2MB bf16). wC for all 4 local batch
chunks is computed LOCALLY — no collective. (The gpsimd AllGather of the
pure D-sharded variant measured a fixed ~55us CC-handshake latency that
froze the wC-gated pipeline; the pure B-sharded variant paid 33.5MB of
replicated vocab DMA. This grid pays 8.4MB vocab and zero collectives.)

Kernel structure (from trace analysis of v2-v5):
  - fp8(e4m3) matmul in DoubleRow perf mode; attn scaled x128 on device
    and stored fp8 (measured ~263ns per 512-wide DR matmul).
  - stem subtracted IN PSUM via an identity matmul (lhsT = -128*I fp8,
    rhs = resident bf16 stem tile): psum = 128*(affix - stem), so the
    elementwise tail is only
      ScalarE: delta = psum * 1/128      (PSUM->SBUF bf16 drain)
      DVE:     prod  = delta * wC        (bf16 2x)
      DVE:     out   = prod + stem       (bf16 2x)
  - batch-chunk outer loop with the full 8.4MB fp8 vocab resident in
    SBUF (64KB/partition) and 8-16KB per-partition DMA lines (the DMA
    engines are packet-overhead limited at ~20GB/s each).
  - pivot path per chunk: one fused exp on ScalarE, TT-add tree + small
    reduce for the group sums, weighted STT chain + scan on DVE; emitted
    one chunk ahead of the main-loop segment that consumes it, so the
    DVE-heavy chain work spreads across the whole kernel.
"""

import os
import sys

import numpy as np

for _p in ("/opt/trn_rl_repo",):
    if os.path.isdir(_p) and _p not in sys.path:
        sys.path.append(_p)

import concourse.bass as bass  # noqa: E402
import concourse.tile as tile  # noqa: E402
from concourse import bacc, mybir  # noqa: E402
from concourse.bass import ts  # noqa: E402
from concourse.bass_utils import run_bass_kernel_spmd  # noqa: E402
from concourse.masks import make_identity  # noqa: E402

import ml_dtypes  # noqa: E402

B, D, N, DM, V = 1024, 256, 256, 128, 512
NCORES = 8
NBG, NDG = 2, 4             # core grid: batch groups x d groups
BLOC = B // NBG             # 512 batches per core
NBC = BLOC // 128           # 4 local batch chunks
DLOC = D // NDG             # 64 d-values per core
NDB = 2                     # d-blocks per core
DB = DLOC // NDB            # 32 d-values per block
DN = DB * N                 # 8192 free elems per (chunk, d-block) tile
PSW = 1024                  # psum tile = 2 banks
QW = 2048                   # stem DMA slice
NT = DN // PSW              # 4 h-tiles
SCALE = 128.0               # attn pre-scale so fp8 subnormals don't bite

F32 = mybir.dt.float32
F32R = mybir.dt.float32r
BF16 = mybir.dt.bfloat16
FP8 = mybir.dt.float8e4
EXP = mybir.ActivationFunctionType.Exp
COPY = mybir.ActivationFunctionType.Copy
ALU = mybir.AluOpType
DR = mybir.MatmulPerfMode.DoubleRow

LAST_RESULT = None

_CACHE = {}


def _build():
    key = 0
    if key in _CACHE:
        return _CACHE[key]

    nc = bacc.Bacc("TRN2", target_bir_lowering=False, debug=False,
                   num_devices=NCORES)

    stem_d = nc.dram_tensor("stem", [BLOC, DLOC, N], BF16, kind="ExternalInput").ap()
    vocab_d = nc.dram_tensor("vocab", [V, DLOC, N], FP8, kind="ExternalInput").ap()
    mor_d = nc.dram_tensor("morpho", [BLOC, DM], F32, kind="ExternalInput").ap()
    waff_d = nc.dram_tensor("waffix", [DM, V], F32R, kind="ExternalInput").ap()
    pv_d = nc.dram_tensor("pivot", [2, 2, BLOC, 5, N], BF16, kind="ExternalInput").ap()
    abf_d = nc.dram_tensor("abf", [1, 9], F32, kind="ExternalInput").ap()
    out_d = nc.dram_tensor("out", [BLOC, DLOC, N], BF16, kind="ExternalOutput").ap()

    from contextlib import ExitStack

    with tile.TileContext(nc) as tc, ExitStack() as ctx:
        const = ctx.enter_context(tc.tile_pool(name="const", bufs=1))

        ident = const.tile([128, 128], F32)
        make_identity(nc, ident[:, :])
        negI = const.tile([128, 128], FP8)
        nc.scalar.mul(negI[:, :], ident[:, :], -SCALE)

        attnT = const.tile([128, 4, BLOC], FP8)    # [v_part, vc, local b]
        w_bcast = const.tile([128, 20], F32)
        wsb = const.tile([128, V], F32R)           # W_affix resident
        mor_sb = const.tile([128, NBC, DM], F32)
        wc_sb = const.tile([128, NBC, N], BF16)    # local wC per chunk

        # ---------- DMAs: wC path on the Activation HWDGE queue ----------
        small = ctx.enter_context(tc.tile_pool(name="small", bufs=1))
        pvp = ctx.enter_context(tc.tile_pool(name="pv", bufs=1))
        abf = small.tile([1, 9], F32)
        nc.scalar.dma_start(abf[0:1, :], abf_d[:, :])
        pvt = []
        for bc in range(NBC):
            pvc = pvp.tile([128, 4, 5, N], BF16, tag="pvin", name=f"pv{bc}")
            for ij in range(4):
                i, j = divmod(ij, 2)
                nc.scalar.dma_start(pvc[:, ij, :, :],
                                    pv_d[i, j, ts(bc, 128), :, :])
            pvt.append(pvc)
        for bc in range(NBC):
            nc.scalar.dma_start(mor_sb[:, bc, :], mor_d[ts(bc, 128), :])
        nc.scalar.dma_start(wsb[:, :], waff_d[:, :])

        # ---------- mixture weights ----------
        eabf = small.tile([1, 9], F32)
        sums = small.tile([1, 3], F32)
        nc.scalar.activation(eabf[0:1, 0:2], abf[0:1, 0:2], EXP, accum_out=sums[0:1, 0:1])
        nc.scalar.activation(eabf[0:1, 2:4], abf[0:1, 2:4], EXP, accum_out=sums[0:1, 1:2])
        nc.scalar.activation(eabf[0:1, 4:9], abf[0:1, 4:9], EXP, accum_out=sums[0:1, 2:3])
        rsum = small.tile([1, 3], F32)
        nc.vector.reciprocal(rsum[0:1, :], sums[0:1, :])
        t4 = small.tile([1, 4], F32)
        nc.vector.tensor_mul(
            t4[0:1, :].rearrange("p (i j) -> p i j", i=2),
            eabf[0:1, 0:2].rearrange("p (i j) -> p i j", j=1).to_broadcast((1, 2, 2)),
            eabf[0:1, 2:4].rearrange("p (i j) -> p i j", i=1).to_broadcast((1, 2, 2)),
        )
        t20 = small.tile([1, 20], F32)
        nc.vector.tensor_mul(
            t20[0:1, :].rearrange("p (g k) -> p g k", g=4),
            t4[0:1, :].rearrange("p (g k) -> p g k", k=1).to_broadcast((1, 4, 5)),
            eabf[0:1, 4:9].rearrange("p (g k) -> p g k", g=1).to_broadcast((1, 4, 5)),
        )
        rr = small.tile([1, 1], F32)
        nc.vector.tensor_mul(rr[0:1, :], rsum[0:1, 0:1], rsum[0:1, 1:2])
        rrr = small.tile([1, 1], F32)
        nc.vector.tensor_mul(rrr[0:1, :], rr[0:1, :], rsum[0:1, 2:3])
        w20 = small.tile([1, 20], F32)
        nc.vector.tensor_scalar_mul(w20[0:1, :], t20[0:1, :], rrr[0:1, 0:1])
        nc.gpsimd.partition_broadcast(w_bcast[:, :], w20[0:1, :])

        # pivot softmax+mix for one chunk; emitted per-chunk, interleaved
        # with the main loop (needs no PSUM, so it runs behind the MMs)
        def pivot_chunk(bc):
            pvc = pvt[bc]
            pvE = pvp.tile([128, 20, N], BF16, tag="pvE", name=f"pvE{bc}")
            nc.scalar.activation(
                pvE[:, :, :].rearrange("p (ij k) n -> p ij k n", ij=4),
                pvc[:, :, :, :], EXP)
            # group sums: TT-add tree (2x-capable) + small reduce (the direct
            # [128,20,256] TENSOR_REDUCE measured 1x = 5.5us)
            t1 = pvp.tile([128, 20, 128], BF16, tag="t1", name=f"t1{bc}")
            nc.vector.tensor_add(t1[:, :, :], pvE[:, :, 0:128], pvE[:, :, 128:256])
            t2 = pvp.tile([128, 20, 64], BF16, tag="t2", name=f"t2{bc}")
            nc.vector.tensor_add(t2[:, :, :], t1[:, :, 0:64], t1[:, :, 64:128])
            sP = pvp.tile([128, 20, 1], F32, tag="sP", name=f"sP{bc}")
            nc.vector.reduce_sum(sP[:, :, :], t2[:, :, :],
                                 axis=mybir.AxisListType.X)
            rP = pvp.tile([128, 20], F32, tag="rP", name=f"rP{bc}")
            nc.vector.reciprocal(rP[:, :], sP[:, :, 0])
            rPw = pvp.tile([128, 20], F32, tag="rPw", name=f"rPw{bc}")
            nc.vector.tensor_mul(rPw[:, :], rP[:, :], w_bcast[:, :])
            accA = pvp.tile([128, N], BF16, tag="accA", name=f"aA{bc}")
            accB = pvp.tile([128, N], BF16, tag="accB", name=f"aB{bc}")
            nc.vector.tensor_scalar_mul(accA[:, :], pvE[:, 0, :], rPw[:, 0:1])
            cur, nxt = accA, accB
            for g in range(1, 20):
                nc.vector.scalar_tensor_tensor(
                    out=nxt[:, :], in0=pvE[:, g, :], scalar=rPw[:, g:g + 1],
                    in1=cur[:, :], op0=ALU.mult, op1=ALU.add,
                )
                cur, nxt = nxt, cur
            nc.vector.tensor_tensor_scan(
                wc_sb[:, bc, :], data0=cur[:, :], data1=cur[:, :], initial=0.0,
                op0=ALU.add, op1=ALU.bypass,
            )

        # ---------- attention (4 local chunks) ----------
        bp = ctx.enter_context(tc.tile_pool(name="attn", bufs=2))
        psB = tc.alloc_tile_pool(name="psB", bufs=2, space="PSUM")
        psT = tc.alloc_tile_pool(name="psT", bufs=1, space="PSUM")

        for bc in range(NBC):
            morT_ps = psB.tile([128, DM], F32, tag="morT_ps", name=f"mtp{bc}")
            nc.tensor.transpose(morT_ps[:, :], mor_sb[:, bc, :], ident[:, :])
            morT = bp.tile([128, DM], F32R, tag="morT", name=f"mt{bc}")
            nc.vector.tensor_copy(morT[:, :], morT_ps[:, :])
            lg_ps = psB.tile([128, V], F32, tag="lg_ps", name=f"lgp{bc}")
            nc.tensor.matmul(lg_ps[:, :], lhsT=morT[:, :], rhs=wsb[:, :],
                             start=True, stop=True)
            E = bp.tile([128, V], F32, tag="E", name=f"E{bc}")
            sE = bp.tile([128, 1], F32, tag="sE", name=f"sE{bc}")
            nc.scalar.activation(E[:, :], lg_ps[:, :], EXP, accum_out=sE[:, :])
            rE = bp.tile([128, 1], F32, tag="rE", name=f"rE{bc}")
            nc.vector.reciprocal(rE[:, :], sE[:, :])
            rEs = bp.tile([128, 1], F32, tag="rEs", name=f"rs{bc}")
            nc.vector.tensor_scalar_mul(rEs[:, :], rE[:, :], SCALE)
            attn = bp.tile([128, V], F32, tag="at", name=f"at{bc}")
            nc.vector.tensor_scalar_mul(attn[:, :], E[:, :], rEs[:, 0:1])
            tp = psT.tile([128, 4, 512], F32, tag="tp", name=f"tp{bc}")
            for vc in range(4):
                nc.tensor.transpose(tp[:, vc, 0:128], attn[:, ts(vc, 128)],
                                    ident[:, :])
            nc.scalar.copy(attnT[:, :, ts(bc, 128)], tp[:, :, 0:128])
        psT.release()
        psB.release()

        # ---------- main loop: d-block outer, batch-chunk inner ----------
        vqp = ctx.enter_context(tc.tile_pool(name="vq", bufs=1))
        stp = ctx.enter_context(tc.tile_pool(name="stem", bufs=2))
        otp = ctx.enter_context(tc.tile_pool(name="outp", bufs=2))
        dlp = ctx.enter_context(tc.tile_pool(name="delta", bufs=6))
        prp = ctx.enter_context(tc.tile_pool(name="prod", bufs=2))
        psD = ctx.enter_context(tc.tile_pool(name="psD", bufs=4, space="PSUM"))

        # full vocab resident (64KB/partition); bc-outer order spreads the
        # pivot-chain DVE work evenly across the whole kernel
        vq = vqp.tile([128, 4, NDB, DN], FP8)
        for vc in range(4):
            for db in range(NDB):
                nc.sync.dma_start(
                    vq[:, vc, db, :],
                    vocab_d[ts(vc, 128), ts(db, DB), :].rearrange("p d n -> p (d n)"),
                )
        pivot_chunk(0)
        for bc in range(NBC):
            for db in range(NDB):
                if db == 1 and bc + 1 < NBC:
                    pivot_chunk(bc + 1)
                stem_t = stp.tile([128, DN], BF16)
                for q in range(DN // QW):
                    nc.sync.dma_start(
                        stem_t[:, ts(q, QW)],
                        stem_d[ts(bc, 128),
                               bass.ds(db * DB + q * (QW // N), QW // N), :]
                        .rearrange("p d n -> p (d n)"),
                    )
                out_t = otp.tile([128, DN], BF16)
                for h in range(NT):
                    col0 = h * PSW
                    ps = psD.tile([128, PSW], F32)
                    pseq = (0, 1) if h % 2 == 0 else (1, 0)
                    for pi, p in enumerate(pseq):
                        for t in range(PSW // 512):
                            c = col0 + t * 512
                            nc.tensor.matmul(
                                ps[:, ts(t, 512)],
                                lhsT=attnT[:, 2 * p:2 * p + 2, ts(bc, 128)],
                                rhs=vq[:, 2 * p:2 * p + 2, db, c:c + 512],
                                start=(pi == 0), stop=False,
                                perf_mode=DR,
                            )
                    # subtract SCALE*stem in PSUM: psum = SCALE*(affix-stem)
                    for t in range(PSW // 512):
                        c = col0 + t * 512
                        nc.tensor.matmul(
                            ps[:, ts(t, 512)],
                            lhsT=negI[:, :],
                            rhs=stem_t[:, c:c + 512],
                            start=False, stop=True,
                        )
                    delta_t = dlp.tile([128, PSW], BF16)
                    nc.scalar.activation(delta_t[:, :], ps[:, :], COPY,
                                         scale=1.0 / SCALE)
                    prod = prp.tile([128, PSW], BF16)
                    nc.vector.tensor_mul(
                        prod[:, :].rearrange("p (a n) -> p a n", n=N),
                        delta_t[:, :].rearrange("p (a n) -> p a n", n=N),
                        wc_sb[:, bc:bc + 1, :].to_broadcast((128, PSW // N, N)),
                    )
                    nc.vector.tensor_add(out_t[:, col0:col0 + PSW], prod[:, :],
                                         stem_t[:, col0:col0 + PSW])
                for q in range(2):
                    nc.sync.dma_start(
                        out_d[ts(bc, 128),
                              bass.ds(db * DB + q * (DB // 2), DB // 2), :]
                        .rearrange("p d n -> p (d n)"),
                        out_t[:, ts(q, DN // 2)],
                    )

    nc.compile()
    _CACHE[key] = nc
    return nc


def kernel(stem_form, morphosyn, pivot_logits, W_affix, affix_vocab,
           alpha, beta, phi, max_len):
    global LAST_RESULT
    stem_form = np.ascontiguousarray(np.asarray(stem_form, dtype=np.float32))
    morphosyn = np.ascontiguousarray(np.asarray(morphosyn, dtype=np.float32))
    pivot_logits = np.ascontiguousarray(np.asarray(pivot_logits, dtype=np.float32))
    W_affix = np.ascontiguousarray(np.asarray(W_affix, dtype=np.float32))
    affix_vocab = np.ascontiguousarray(np.asarray(affix_vocab, dtype=np.float32))
    abf = np.concatenate([
        np.asarray(alpha, np.float32).ravel(),
        np.asarray(beta, np.float32).ravel(),
        np.asarray(phi, np.float32).ravel(),
    ]).reshape(1, 9)

    nc = _build()

    stem_np = stem_form.astype(ml_dtypes.bfloat16)
    vocab_np = affix_vocab.astype(ml_dtypes.float8_e4m3)
    pivot_np = pivot_logits.astype(ml_dtypes.bfloat16)

    in_maps = []
    for c in range(NCORES):
        bg, dg = divmod(c, NDG)
        blo, bhi = bg * BLOC, (bg + 1) * BLOC
        dlo, dhi = dg * DLOC, (dg + 1) * DLOC
        in_maps.append({
            "stem": np.ascontiguousarray(stem_np[blo:bhi, dlo:dhi, :]),
            "vocab": np.ascontiguousarray(vocab_np[:, dlo:dhi, :]),
            "morpho": np.ascontiguousarray(morphosyn[blo:bhi]),
            "waffix": W_affix,
            "pivot": np.ascontiguousarray(pivot_np[:, :, blo:bhi, :, :]),
            "abf": abf,
        })

    LAST_RESULT = run_bass_kernel_spmd(nc, in_maps, core_ids=list(range(NCORES)))
    out = np.empty((B, D, N), dtype=np.float32)
    for c in range(NCORES):
        bg, dg = divmod(c, NDG)
        out[bg * BLOC:(bg + 1) * BLOC, dg * DLOC:(dg + 1) * DLOC, :] = \
            LAST_RESULT.results[c]["out"].astype(np.float32)
    return np.ascontiguousarray(out)
